# revision 1
# baseline (speedup 1.0000x reference)
"""Trainium2 Bass kernel for nn_MACEConvolutionLayer.

Strategy (8 NeuronCores, no collectives):
  - Edges are sharded by DESTINATION node range (1250 nodes/core), sorted and
    packed into 10 windows of 128 nodes x 1024 edge slots per core. Each core
    computes messages for its edge shard and segment-sums them into its own
    node shard via one-hot matmuls on the tensor engine.
  - Per-edge bilinear (radial-MLP features x embedded source scalars) and the
    per-node equivariant tensor products are computed via a monomial scheme:
    DVE forms per-sample outer-product monomials with broadcast access
    patterns, PE transposes the monomial tiles and contracts them against
    host-precomputed combined weight matrices (Clebsch-Gordan x TP weights,
    with channel-mixing/combination linears folded in).

Feature layout on device is kappa-major: col(l, i, u) = LOFF[l] + i*32 + u.
"""
import sys, os

sys.path.insert(0, '/opt/trn_rl_repo')

import numpy as np
import ml_dtypes

MUL = 32
DIMS = (1, 3, 5)
HID = 288
N_NODES = 10000
N_EDGES = 64000
RHID = 64
SQM = float(np.sqrt(MUL))
LOFF = [0, 32, 128]
SOFF = [0, 1, 4]
PATHS_FULL = [(0,0,0),(0,1,1),(0,2,2),(1,0,1),(1,1,0),(1,1,2),(1,2,1),(2,0,2),(2,1,1),(2,2,0),(2,2,2)]
O2_UVW = [(0,1,1),(0,2,2),(1,2,1)]
O2_UVU = [(0,0,0),(1,1,0),(1,1,2),(2,2,0),(2,2,2)]

N_CORES = 8
NODES_PER_CORE = 1250
WIN = 128
N_WIN = 10
ESLOT = 1024
E_PAD = N_WIN * ESLOT   # 10240
ET_PER_WIN = ESLOT // 128  # 8
BF = ml_dtypes.bfloat16

# output psum blocks (cols of the 288-wide kappa-major vector)
MBLK = [(0, 128), (128, 256), (256, 288)]


def cg_np():
    s2, s3, s5, s6 = map(np.sqrt, (2.0, 3.0, 5.0, 6.0))
    B = np.zeros((5, 3, 3))
    B[0, 0, 1] = B[0, 1, 0] = 1 / s2
    B[1, 1, 2] = B[1, 2, 1] = 1 / s2
    B[2] = np.diag([-1.0, -1.0, 2.0]) / s6
    B[3, 0, 2] = B[3, 2, 0] = 1 / s2
    B[4] = np.diag([1.0, -1.0, 0.0]) / s2
    C = {}
    C[(0, 0, 0)] = np.ones((1, 1, 1))
    C[(0, 1, 1)] = (np.eye(3) / s3)[None]
    C[(1, 0, 1)] = np.transpose(C[(0, 1, 1)], (1, 0, 2))
    C[(0, 2, 2)] = (np.eye(5) / s5)[None]
    C[(2, 0, 2)] = np.transpose(C[(0, 2, 2)], (1, 0, 2))
    C[(1, 1, 0)] = (np.eye(3) / s3)[:, :, None]
    C[(1, 1, 2)] = np.transpose(B, (1, 2, 0)) / s5
    C[(1, 2, 1)] = np.transpose(B, (1, 0, 2)) / s5
    C[(2, 1, 1)] = B / s5
    C[(2, 2, 0)] = (np.eye(5) / s5)[:, :, None]
    T = np.einsum('aij,bjk,cki->abc', B, B, B)
    C[(2, 2, 2)] = T / np.linalg.norm(T)
    return C

CG = cg_np()
PATH_LIST_O2 = O2_UVW + O2_UVU  # (i,j,k) in folded order


def support_pairs(path_ijk):
    d = {}
    for pi, (li, lj, lk) in enumerate(path_ijk):
        C = CG[(li, lj, lk)]
        for iloc in range(DIMS[li]):
            for jloc in range(DIMS[lj]):
                if np.any(np.abs(C[iloc, jloc, :]) > 1e-12):
                    d.setdefault(((li, iloc), (lj, jloc)), []).append((pi, iloc, jloc))
    return d


def build_mono_blocks_sym(path_ijk):
    d = support_pairs(path_ijk)
    blocks = {}
    for (I, J), lst in d.items():
        key = (min(I, J), max(I, J))
        swap = I > J
        for (pi, iloc, jloc) in lst:
            blocks.setdefault(key, []).append((pi, iloc, jloc, swap))
    return [(I, J, c) for (I, J), c in sorted(blocks.items())]


def build_mono_blocks(path_ijk):
    d = support_pairs(path_ijk)
    return [(I, J, [(pi, i, j, False) for (pi, i, j) in lst]) for (I, J), lst in sorted(d.items())]


def omega_for_block(path_ijk, weights, I, J, contribs):
    """[1024 (u-major,v-fast), 288] kappa-major outputs."""
    Om = np.zeros((MUL * MUL, HID))
    for (pi, iloc, jloc, swap) in contribs:
        li, lj, lk = path_ijk[pi]
        W = weights[pi]
        C = CG[(li, lj, lk)]
        for kap in range(DIMS[lk]):
            c = C[iloc, jloc, kap]
            if abs(c) < 1e-12:
                continue
            c0 = LOFF[lk] + kap * 32
            Wm = W if not swap else np.transpose(W, (1, 0, 2))
            Om[:, c0:c0 + 32] += c * Wm.reshape(MUL * MUL, MUL)
    return Om


# ---------------------------------------------------------------------------
# static plan: monomial blocks + emission structure (depends only on CG)
# ---------------------------------------------------------------------------

class Plan:
    pass


GAPTOL = 4

def _emissions_for_pair(mq, mm_):
    """Emissions in the combined 576-col space: q outputs at cols [0:288),
    msg outputs at [288:576). mq/mm_: [1024, 288] bool masks (or None).
    Returns list over kchunk of list of (c0, c1) in combined space."""
    out = []
    for kc in range(8):
        used32 = []
        for mask, base in ((mq, 0), (mm_, 9)):
            if mask is None:
                continue
            sub = mask[kc * 128:(kc + 1) * 128]
            for g in range(9):
                if np.any(sub[:, g * 32:(g + 1) * 32]):
                    used32.append(base + g)
        emis = []
        if used32:
            runs = [[used32[0], used32[0] + 1]]
            for g in used32[1:]:
                if g - runs[-1][1] <= GAPTOL:
                    runs[-1][1] = g + 1
                else:
                    runs.append([g, g + 1])
            for (ga, gb) in runs:
                emis.append((ga * 32, gb * 32))
        out.append(emis)
    return out


def build_plan():
    p = Plan()
    p.aa_blocks = build_mono_blocks_sym(PATHS_FULL + PATH_LIST_O2)
    p.qa_blocks = build_mono_blocks(PATHS_FULL)
    n3a = len(PATHS_FULL)
    ones_a = [np.ones((MUL, MUL, MUL)) for _ in PATHS_FULL]
    ones_o2 = [np.ones((MUL, MUL, MUL)) for _ in PATH_LIST_O2]

    # per aa block: contribs split into q-part (o3a) and msg-part (o2)
    p.aa = []
    for (I, J, contribs) in p.aa_blocks:
        cq = [(pi, i, j, s) for (pi, i, j, s) in contribs if pi < n3a]
        cm = [(pi - n3a, i, j, s) for (pi, i, j, s) in contribs if pi >= n3a]
        mq = omega_for_block(PATHS_FULL, ones_a, I, J, cq) != 0 if cq else None
        mm_ = omega_for_block(PATH_LIST_O2, ones_o2, I, J, cm) != 0 if cm else None
        em = _emissions_for_pair(mq, mm_)
        p.aa.append((I, J, cq, cm, em))
    p.qa = []
    for (I, J, contribs) in p.qa_blocks:
        mm_ = omega_for_block(PATHS_FULL, ones_a, I, J, contribs) != 0
        em = _emissions_for_pair(None, mm_)
        p.qa.append((I, J, contribs, em))

    # assign omega column offsets; emission = (c0, c1, om_off) combined space
    off = 0
    p.aa_emi = []
    for (I, J, cq, cm, em) in p.aa:
        bk = []
        for kc in range(8):
            lst = []
            for (c0, c1) in em[kc]:
                lst.append((c0, c1, off))
                off += c1 - c0
            bk.append(lst)
        p.aa_emi.append(bk)
    p.qa_emi = []
    for (I, J, contribs, em) in p.qa:
        bk = []
        for kc in range(8):
            lst = []
            for (c0, c1) in em[kc]:
                lst.append((c0, c1, off))
                off += c1 - c0
            bk.append(lst)
        p.qa_emi.append(bk)
    p.totc = off
    n_emi = sum(len(l) for bk in p.aa_emi + p.qa_emi for l in bk)
    p.n_emi = n_emi
    return p


def pack_omega(plan, Wfold):
    """Fill the packed omega array [128, totc] (bf16) from folded weights.
    Emission col-ranges live in the combined 576-col space (q | msg)."""
    W3a = Wfold['o3a_w']; Wo2 = Wfold['o2_w']; W3b = Wfold['o3b_w']
    om = np.zeros((128, plan.totc), np.float32)
    for bi, (I, J, cq, cm, em) in enumerate(plan.aa):
        Om = np.zeros((MUL * MUL, 2 * HID))
        if cq:
            Om[:, :HID] = omega_for_block(PATHS_FULL, W3a, I, J, cq)
        if cm:
            Om[:, HID:] = omega_for_block(PATH_LIST_O2, Wo2, I, J, cm)
        for kc in range(8):
            for (c0, c1, off) in plan.aa_emi[bi][kc]:
                om[:, off:off + (c1 - c0)] = Om[kc * 128:(kc + 1) * 128, c0:c1]
    for bi, (I, J, contribs, em) in enumerate(plan.qa):
        Om = np.zeros((MUL * MUL, 2 * HID))
        Om[:, HID:] = omega_for_block(PATHS_FULL, W3b, I, J, contribs)
        for kc in range(8):
            for (c0, c1, off) in plan.qa_emi[bi][kc]:
                om[:, off:off + (c1 - c0)] = Om[kc * 128:(kc + 1) * 128, c0:c1]
    return om.astype(BF)


def fold_weights(inp):
    f8 = np.float64
    mix_w = inp['mix_w'].astype(f8); comb_w = inp['comb_w'].astype(f8)
    M = np.einsum('olux,olxw->oluw', mix_w, comb_w) / MUL
    W1eff = np.einsum('lux,lxw->luw', inp['lin_o1'].astype(f8), M[0]) / SQM
    o2_w = []
    for pp, (i, j, k) in enumerate(O2_UVW):
        o2_w.append(np.einsum('uvx,xw->uvw', inp['o2_uvw'][pp].astype(f8) / MUL, M[1][k]))
    for pp, (i, j, k) in enumerate(O2_UVU):
        o2_w.append(np.einsum('uv,uw->uvw', inp['o2_uvu'][pp].astype(f8), M[1][k]) / SQM)
    o3a_w = [inp['o3a_uvw'][pp].astype(f8) / MUL for pp in range(len(PATHS_FULL))]
    o3b_w = [np.einsum('uvx,xw->uvw', inp['o3b_uvw'][pp].astype(f8) / MUL, M[2][k])
             for pp, (i, j, k) in enumerate(PATHS_FULL)]
    aw = inp['a_w'].astype(f8).reshape(RHID, 3, MUL, MUL)
    ab = inp['a_b'].astype(f8).reshape(3, MUL, MUL)
    scale = np.array([1.0 / np.sqrt(d) for d in DIMS]) / SQM
    aw = aw * scale[None, :, None, None]
    ab = ab * scale[:, None, None]
    A2 = np.transpose(aw, (0, 2, 1, 3)).reshape(RHID * MUL, 3 * MUL)
    B2 = np.transpose(ab, (1, 0, 2)).reshape(MUL, 3 * MUL)
    # c1 block-diagonal omegas per aT chunk (kappa-major rows/cols)
    omc1 = np.zeros((HID, HID))
    for l in range(3):
        for i in range(DIMS[l]):
            c = LOFF[l] + i * 32
            omc1[c:c + 32, c:c + 32] = W1eff[l]
    return dict(
        o3a_w=o3a_w, o2_w=o2_w, o3b_w=o3b_w,
        omc1=omc1, omself=inp['self_w'].astype(f8) / SQM,
        emb=inp['emb_w'].astype(f8) / SQM,
        A2=A2, B2=B2,
        r_w1=inp['r_w1'].astype(np.float32), r_b1=inp['r_b1'].astype(np.float32),
        r_w2=inp['r_w2'].astype(np.float32), r_b2=inp['r_b2'].astype(np.float32),
        r_w3=inp['r_w3'].astype(np.float32), r_b3=inp['r_b3'].astype(np.float32),
    )


def pack_edges(inp):
    src = np.asarray(inp['edge_index'][0]).astype(np.int64)
    dst = np.asarray(inp['edge_index'][1]).astype(np.int64)
    sh = np.asarray(inp['edge_sh'], dtype=np.float32)
    rad = np.asarray(inp['edge_radial_embedding'], dtype=np.float32)
    attr = np.asarray(inp['edge_attr'], dtype=np.float32)
    nf = np.asarray(inp['node_features'], dtype=np.float32)
    order = np.argsort(dst, kind='stable')
    dst_s = dst[order]; src_s = src[order]
    cores = []
    for c in range(N_CORES):
        lo = c * NODES_PER_CORE
        rinT = np.zeros((24, E_PAD), np.float32)
        nfsT = np.zeros((MUL, E_PAD), np.float32)
        shdv = np.zeros((E_PAD, 12), np.float32)   # 0..8 sh, 9 dst_local, 10 valid
        for w in range(N_WIN):
            nlo = lo + w * WIN
            nhi = min(lo + (w + 1) * WIN, lo + NODES_PER_CORE)
            a = np.searchsorted(dst_s, nlo); b = np.searchsorted(dst_s, nhi)
            idx = order[a:b]
            n = b - a
            assert n <= ESLOT, f"window overflow {n}"
            s = w * ESLOT
            rinT[:8, s:s + n] = rad[idx].T
            rinT[8:, s:s + n] = attr[idx].T
            nfsT[:, s:s + n] = nf[src[idx]].T
            shdv[s:s + n, :9] = sh[idx]
            shdv[s:s + n, 9] = (dst[idx] - nlo).astype(np.float32)
            shdv[s:s + n, 10] = 1.0
        nfT = np.zeros((MUL, N_WIN * WIN), BF)
        nfT[:, :NODES_PER_CORE] = nf[lo:lo + NODES_PER_CORE].T.astype(BF)
        cores.append(dict(rinT=rinT, nfsT=nfsT, shdv=shdv, nfT=nfT))
    return cores


def ref_from_kap(x_kap):
    out = np.empty_like(x_kap)
    for l, d in enumerate(DIMS):
        blk = x_kap[:, LOFF[l]:LOFF[l] + 32 * d].reshape(-1, d, 32)
        out[:, LOFF[l]:LOFF[l] + 32 * d] = np.transpose(blk, (0, 2, 1)).reshape(-1, 32 * d)
    return out


# ---------------------------------------------------------------------------
# device kernel
# ---------------------------------------------------------------------------

_NC_CACHE = {}
LAST_RESULT = None


def build_nc(plan):
    import concourse.bass as bass
    import concourse.bacc as bacc
    import concourse.mybir as mybir
    import concourse.tile as tile

    f32 = mybir.dt.float32
    bf16 = mybir.dt.bfloat16
    AL = mybir.AluOpType
    AF = mybir.ActivationFunctionType

    nc = bacc.Bacc(None)
    P = 128

    # ---- dram parameters
    rinT_d = nc.declare_dram_parameter("rinT", [24, E_PAD], f32, isOutput=False)
    nfsT_d = nc.declare_dram_parameter("nfsT", [32, E_PAD], f32, isOutput=False)
    shdv_d = nc.declare_dram_parameter("shdv", [E_PAD, 12], f32, isOutput=False)
    nfT_d = nc.declare_dram_parameter("nfT", [32, N_WIN * WIN], bf16, isOutput=False)
    omega_d = nc.declare_dram_parameter("omega", [P, plan.totc], bf16, isOutput=False)
    a2_d = nc.declare_dram_parameter("a2", [P, 16 * 96], bf16, isOutput=False)
    b2_d = nc.declare_dram_parameter("b2", [32, 96], bf16, isOutput=False)
    omc1_d = nc.declare_dram_parameter("omc1", [P, HID], bf16, isOutput=False)  # packed: cols 0:128 chunk0, 128:256 chunk1, 256:288 chunk2 (rows 0:32)
    omself_d = nc.declare_dram_parameter("omself", [32, 32], bf16, isOutput=False)
    rw1_d = nc.declare_dram_parameter("rw1", [24, 64], f32, isOutput=False)
    rw2_d = nc.declare_dram_parameter("rw2", [64, 64], f32, isOutput=False)
    rw3_d = nc.declare_dram_parameter("rw3", [64, 64], f32, isOutput=False)
    rb1_d = nc.declare_dram_parameter("rb1", [64, 1], f32, isOutput=False)
    rb2_d = nc.declare_dram_parameter("rb2", [64, 1], f32, isOutput=False)
    emb_d = nc.declare_dram_parameter("emb", [32, 32], f32, isOutput=False)
    iota_d = nc.declare_dram_parameter("iota", [P, P], f32, isOutput=False)
    identb_d = nc.declare_dram_parameter("identb", [P, P], bf16, isOutput=False)
    zer_d = nc.declare_dram_parameter("zer", [1, HID], bf16, isOutput=False)
    out_d = nc.declare_dram_parameter("out", [N_WIN * WIN, HID], f32, isOutput=True)

    from contextlib import ExitStack
    with tile.TileContext(nc) as tc, ExitStack() as es:
        cst = es.enter_context(tc.tile_pool(name="cst", bufs=1))
        sb = es.enter_context(tc.tile_pool(name="sb", bufs=2))
        sb3 = es.enter_context(tc.tile_pool(name="sb3", bufs=3))
        ps = es.enter_context(tc.tile_pool(name="ps", bufs=1, space="PSUM"))
        ps2 = es.enter_context(tc.tile_pool(name="ps2", bufs=2, space="PSUM"))
        ps3 = es.enter_context(tc.tile_pool(name="ps3", bufs=3, space="PSUM"))

        # ---- constants into SBUF
        omega = cst.tile([P, plan.totc], bf16)
        nc.sync.dma_start(out=omega[:], in_=omega_d[:])
        a2 = cst.tile([P, 16 * 96], bf16)
        nc.sync.dma_start(out=a2[:], in_=a2_d[:])
        b2 = cst.tile([32, 96], bf16)
        nc.sync.dma_start(out=b2[:], in_=b2_d[:])
        omc1 = cst.tile([P, HID], bf16)
        nc.sync.dma_start(out=omc1[:], in_=omc1_d[:])
        omself = cst.tile([32, 32], bf16)
        nc.sync.dma_start(out=omself[:], in_=omself_d[:])
        rw1 = cst.tile([24, 64], f32); nc.sync.dma_start(out=rw1[:], in_=rw1_d[:])
        rw2 = cst.tile([64, 64], f32); nc.sync.dma_start(out=rw2[:], in_=rw2_d[:])
        rw3 = cst.tile([64, 64], f32); nc.sync.dma_start(out=rw3[:], in_=rw3_d[:])
        rb1 = cst.tile([64, 1], f32); nc.sync.dma_start(out=rb1[:], in_=rb1_d[:])
        rb2 = cst.tile([64, 1], f32); nc.sync.dma_start(out=rb2[:], in_=rb2_d[:])
        emb = cst.tile([32, 32], f32); nc.sync.dma_start(out=emb[:], in_=emb_d[:])
        iota = cst.tile([P, P], f32); nc.sync.dma_start(out=iota[:], in_=iota_d[:])
        identb = cst.tile([P, P], bf16); nc.sync.dma_start(out=identb[:], in_=identb_d[:])
        zer = cst.tile([1, HID], bf16); nc.sync.dma_start(out=zer[:], in_=zer_d[:])
        zer2 = cst.tile([1, 2 * HID], bf16); nc.gpsimd.memset(zer2[:], 0.0)
        nfT = cst.tile([32, N_WIN * WIN], bf16)
        nc.sync.dma_start(out=nfT[:], in_=nfT_d[:])

        # manual psum bank: 2 banks, two alternating 512-col sets
        edgebank = ps.tile([P, 512], f32, space="PSUM")

        def edge_tile(w, j, rin_w, nfs_w):
            """process edge tile j (0..7) of window w; returns msgs, S tiles."""
            t = w * ET_PER_WIN + j
            e0 = t * P
            so = 0  # single psum set
            l1p = edgebank[0:64, so + 0:so + 128]
            l2p = edgebank[0:64, so + 128:so + 256]
            rfp = edgebank[0:128, so + 256:so + 320]
            hp = edgebank[0:128, so + 320:so + 352]
            hTp = edgebank[0:32, so + 352:so + 480]
            mxp = edgebank[0:128, so + 0:so + 96]  # reuses l1 cols after silu

            shdv_t = sb3.tile([P, 12], f32, tag="shdv")
            nc.sync.dma_start(out=shdv_t[:], in_=shdv_d[e0:e0 + P, :])

            rin_t = rin_w[:, j * P:(j + 1) * P]
            nfs_t = nfs_w[:, j * P:(j + 1) * P]
            # radial MLP (feature-major)
            nc.tensor.matmul(out=l1p, lhsT=rw1[:], rhs=rin_t, start=True, stop=True)
            f1 = sb.tile([64, P], f32, tag="f1")
            nc.scalar.activation(out=f1[:], in_=l1p, func=AF.Silu, bias=rb1[:], scale=1.0)
            nc.tensor.matmul(out=l2p, lhsT=rw2[:], rhs=f1[:], start=True, stop=True)
            f2 = sb.tile([64, P], f32, tag="f2")
            nc.scalar.activation(out=f2[:], in_=l2p, func=AF.Silu, bias=rb2[:], scale=1.0)
            # rf (edge-major) = f2.T @ rw3 ; b3 folded into B2
            nc.tensor.matmul(out=rfp, lhsT=f2[:], rhs=rw3[:], start=True, stop=True)
            rfb = sb.tile([P, 64], bf16, tag="rfb")
            nc.vector.tensor_copy(out=rfb[:], in_=rfp)
            # h (edge-major) and hT
            nc.tensor.matmul(out=hp, lhsT=nfs_t, rhs=emb[:], start=True, stop=True)
            hb = sb.tile([P, 32], bf16, tag="hb")
            nc.vector.tensor_copy(out=hb[:], in_=hp)
            nc.tensor.matmul(out=hTp, lhsT=emb[:], rhs=nfs_t, start=True, stop=True)
            hTb = sb.tile([32, P], bf16, tag="hTb")
            nc.vector.tensor_copy(out=hTb[:], in_=hTp)
            # monomials m[e, (r,u)]
            m = sb.tile([P, 2048], bf16, tag="m")
            nc.vector.tensor_tensor(
                out=m[:].rearrange("p (r u) -> p r u", u=32),
                in0=rfb[:][:, :, None].broadcast_to([P, 64, 32]),
                in1=hb[:][:, None, :].broadcast_to([P, 64, 32]),
                op=AL.mult)
            # transpose 16 chunks, pack 4 per psum tile, evacuate
            mT = sb.tile([P, 2048], bf16, tag="mT")
            for g in range(4):
                tp = ps3.tile([P, 512], bf16, space="PSUM", tag="tp")
                for c in range(4):
                    nc.tensor.transpose(out=tp[:, c * P:(c + 1) * P],
                                        in_=m[:, (g * 4 + c) * P:(g * 4 + c + 1) * P],
                                        identity=identb[:])
                if g % 2 == 0:
                    nc.vector.tensor_copy(out=mT[:, g * 512:(g + 1) * 512], in_=tp[:])
                else:
                    nc.scalar.copy(out=mT[:, g * 512:(g + 1) * 512], in_=tp[:])
            # mixed = m @ A2 + h @ B2  (PSUM accumulate)
            for c in range(16):
                nc.tensor.matmul(out=mxp, lhsT=mT[:, c * P:(c + 1) * P],
                                 rhs=a2[:, c * 96:(c + 1) * 96],
                                 start=(c == 0), stop=False)
            nc.tensor.matmul(out=mxp, lhsT=hTb[:], rhs=b2[:], start=False, stop=True)
            # messages: msgs[e, LOFF+i*32+u] = sh[e, SOFF+i] * mixed[e, l*32+u]
            msgs = sb3.tile([P, HID + 1], f32, tag="msgs")
            for l, d in enumerate(DIMS):
                nc.vector.tensor_tensor(
                    out=msgs[:, LOFF[l]:LOFF[l] + 32 * d].rearrange("p (i u) -> p i u", u=32),
                    in0=shdv_t[:, SOFF[l]:SOFF[l] + d][:, :, None].broadcast_to([P, d, 32]),
                    in1=mxp[:, l * 32:(l + 1) * 32][:, None, :].broadcast_to([P, d, 32]),
                    op=AL.mult)
            nc.vector.tensor_copy(out=msgs[:, HID:HID + 1], in_=shdv_t[:, 10:11])
            # one-hot S
            S = sb.tile([P, P], f32, tag="S")
            nc.vector.tensor_tensor(out=S[:], in0=shdv_t[:, 9:10].to_broadcast([P, P]),
                                    in1=iota[:], op=AL.is_equal)
            return msgs, S

        def node_window(w, a_bf, aT):
            """equivariant node phase for window w. a_bf: [128, 288] bf16,
            aT: [128, 288-ish...] transposed chunks tile."""
            qm_ps = ps.tile([P, 2 * HID], f32, space="PSUM", tag="qmps")
            # zero-init accumulator (start=True, split <=512-col matmuls)
            nc.tensor.matmul(out=qm_ps[:, 0:512], lhsT=zer[:, 0:P], rhs=zer2[:, 0:512], start=True, stop=False)
            nc.tensor.matmul(out=qm_ps[:, 512:576], lhsT=zer[:, 0:P], rhs=zer2[:, 512:576], start=True, stop=False)

            def do_pass(blocks, emi_list, x_bf, y_bf):
                for bi, (I, J, *_rest) in enumerate(blocks):
                    cI = LOFF[I[0]] + I[1] * 32
                    cJ = LOFF[J[0]] + J[1] * 32
                    Pm = sb3.tile([P, 1024], bf16, tag="Pm")
                    eng = nc.gpsimd if bi % 3 == 2 else nc.vector
                    eng.tensor_tensor(
                        out=Pm[:].rearrange("p (u v) -> p u v", v=32),
                        in0=x_bf[:, cI:cI + 32][:, :, None].broadcast_to([P, 32, 32]),
                        in1=y_bf[:, cJ:cJ + 32][:, None, :].broadcast_to([P, 32, 32]),
                        op=AL.mult)
                    PT = sb3.tile([P, 1024], bf16, tag="PT")
                    for g in range(2):
                        tp = ps3.tile([P, 512], bf16, space="PSUM", tag="tp")
                        for c in range(4):
                            nc.tensor.transpose(out=tp[:, c * P:(c + 1) * P],
                                                in_=Pm[:, (g * 4 + c) * P:(g * 4 + c + 1) * P],
                                                identity=identb[:])
                        if g == 0:
                            nc.vector.tensor_copy(out=PT[:, :512], in_=tp[:])
                        else:
                            nc.scalar.copy(out=PT[:, 512:], in_=tp[:])
                    for kc in range(8):
                        for (c0, c1, off) in emi_list[bi][kc]:
                            nc.tensor.matmul(out=qm_ps[:, c0:c1],
                                             lhsT=PT[:, kc * P:(kc + 1) * P],
                                             rhs=omega[:, off:off + (c1 - c0)],
                                             start=False, stop=False,
                                             skip_group_check=True)

            do_pass(plan.aa, plan.aa_emi, a_bf, a_bf)
            # evacuate q region to bf16 (aa pass wrote all q contributions)
            q_bf = sb.tile([P, HID], bf16, tag="qbf")
            nc.vector.tensor_copy(out=q_bf[:], in_=qm_ps[:, 0:HID])
            do_pass(plan.qa, plan.qa_emi, q_bf, a_bf)
            # c1: msg += aT-chunks @ omc1-chunks  (block-diagonal linear)
            nc.tensor.matmul(out=qm_ps[:, HID + 0:HID + 128], lhsT=aT[:, 0:P],
                             rhs=omc1[:, 0:128], start=False, stop=False,
                             skip_group_check=True)
            nc.tensor.matmul(out=qm_ps[:, HID + 128:HID + 256], lhsT=aT[:, P:2 * P],
                             rhs=omc1[:, 128:256], start=False, stop=False,
                             skip_group_check=True)
            nc.tensor.matmul(out=qm_ps[:, HID + 256:HID + 288], lhsT=aT[0:32, 2 * P:3 * P],
                             rhs=omc1[0:32, 256:288], start=False,
                             stop=False, skip_group_check=True)
            # self connection (l=0 cols)
            nc.tensor.matmul(out=qm_ps[:, HID:HID + 32], lhsT=nfT[:, w * P:(w + 1) * P],
                             rhs=omself[:], start=False, stop=True,
                             skip_group_check=True)
            # write out (DMA cannot read PSUM; bounce via SBUF)
            out_sb = sb.tile([P, HID], f32, tag="outsb")
            nc.vector.tensor_copy(out=out_sb[:], in_=qm_ps[:, HID:2 * HID])
            nc.sync.dma_start(out=out_d[w * P:(w + 1) * P, :], in_=out_sb[:])

        # ---------------- main loop ----------------
        for w in range(N_WIN):
            rin_w = sb.tile([24, ESLOT], f32, tag="rinw")
            nc.sync.dma_start(out=rin_w[:], in_=rinT_d[:, w * ESLOT:(w + 1) * ESLOT])
            nfs_w = sb.tile([32, ESLOT], f32, tag="nfsw")
            nc.sync.dma_start(out=nfs_w[:], in_=nfsT_d[:, w * ESLOT:(w + 1) * ESLOT])
            wps = ps2.tile([P, HID + 1], f32, space="PSUM", tag="wps")
            for j in range(ET_PER_WIN):
                msgs, S = edge_tile(w, j, rin_w, nfs_w)
                nc.tensor.matmul(out=wps[:], lhsT=S[:], rhs=msgs[:],
                                 start=(j == 0), stop=(j == ET_PER_WIN - 1))
            # normalize: a = wps[:, :288] / max(cnt, 1)
            cnt = sb.tile([P, 1], f32, tag="cnt")
            nc.vector.tensor_scalar_max(out=cnt[:], in0=wps[:, HID:HID + 1], scalar1=1.0)
            rec = sb.tile([P, 1], f32, tag="rec")
            nc.vector.reciprocal(out=rec[:], in_=cnt[:])
            a_bf = sb3.tile([P, HID], bf16, tag="abf")
            nc.vector.tensor_scalar_mul(out=a_bf[:], in0=wps[:, :HID], scalar1=rec[:])
            # aT chunks (for c1): transpose a_bf -> [f, z] 3 chunks packed [128, 384]
            aT = sb3.tile([P, 3 * P], bf16, tag="aT")
            tpa = ps3.tile([P, 512], bf16, space="PSUM", tag="tp")
            nc.tensor.transpose(out=tpa[:, 0:P], in_=a_bf[:, 0:P], identity=identb[:])
            nc.tensor.transpose(out=tpa[:, P:2 * P], in_=a_bf[:, P:2 * P], identity=identb[:])
            nc.tensor.transpose(out=tpa[0:32, 2 * P:3 * P], in_=a_bf[:, 2 * P:HID], identity=identb[:])
            nc.vector.tensor_copy(out=aT[:, 0:2 * P], in_=tpa[:, 0:2 * P])
            nc.vector.tensor_copy(out=aT[0:32, 2 * P:3 * P], in_=tpa[0:32, 2 * P:3 * P])
            node_window(w, a_bf, aT)

    nc.finalize()
    return nc


def _get_nc(plan):
    if 'nc' not in _NC_CACHE:
        _NC_CACHE['nc'] = build_nc(plan)
    return _NC_CACHE['nc']


def kernel(**inputs):
    global LAST_RESULT
    from concourse.bass_utils import run_bass_kernel_spmd

    inp = {k: np.asarray(v) for k, v in inputs.items()}
    plan = build_plan()
    W = fold_weights(inp)
    om = pack_omega(plan, W)

    # A2 packed [128, 16*96]: chunk c cols [c*96:(c+1)*96] holds A2[c*128:(c+1)*128, :]
    A2 = W['A2'].astype(np.float32)
    a2p = np.zeros((128, 16 * 96), np.float32)
    for c in range(16):
        a2p[:, c * 96:(c + 1) * 96] = A2[c * 128:(c + 1) * 128, :]
    # fold b3 into B2: mixed gets (rf0 + b3) x h terms; b3 (x) h part is linear in h
    B2 = W['B2'].astype(np.float64).copy()
    b3 = inp['r_b3'].astype(np.float64)
    for u in range(32):
        B2[u, :] += b3 @ A2[np.arange(RHID) * 32 + u, :].astype(np.float64)
    # omc1 packed [128, 288]
    omc1 = W['omc1']
    omc1p = np.zeros((128, HID), np.float32)
    omc1p[:, 0:128] = omc1[0:128, 0:128]
    omc1p[:, 128:256] = omc1[128:256, 128:256]
    omc1p[0:32, 256:288] = omc1[256:288, 256:288]

    iota = np.broadcast_to(np.arange(128, dtype=np.float32)[None, :], (128, 128)).copy()
    identb = np.eye(128, dtype=np.float32).astype(BF)

    shared = dict(
        omega=om,
        a2=a2p.astype(BF), b2=B2.astype(np.float32).astype(BF),
        omc1=omc1p.astype(BF), omself=W['omself'].astype(np.float32).astype(BF),
        rw1=W['r_w1'], rw2=W['r_w2'], rw3=W['r_w3'],
        rb1=W['r_b1'].reshape(64, 1), rb2=W['r_b2'].reshape(64, 1),
        emb=W['emb'].astype(np.float32),
        iota=iota, identb=identb,
        zer=np.zeros((1, HID), BF),
    )
    cores = pack_edges(inp)
    in_maps = []
    for c in range(N_CORES):
        m = dict(shared)
        m.update(rinT=cores[c]['rinT'], nfsT=cores[c]['nfsT'],
                 shdv=cores[c]['shdv'], nfT=cores[c]['nfT'])
        in_maps.append(m)

    nc = _get_nc(plan)
    res = run_bass_kernel_spmd(nc, in_maps, core_ids=list(range(N_CORES)))
    LAST_RESULT = res
    outs = [res.results[c]['out'][:NODES_PER_CORE] for c in range(N_CORES)]
    out_kap = np.concatenate(outs, axis=0).astype(np.float32)
    return ref_from_kap(out_kap)


if __name__ == "__main__":
    plan = build_plan()
    print(f"aa blocks: {len(plan.aa)}  qa blocks: {len(plan.qa)}")
    print(f"omega cols: {plan.totc}  ({plan.totc * 128 * 2 / 1e6:.1f} MB bf16)")
    print(f"emissions per node-tile: {plan.n_emi}")



# revision 6
# speedup vs baseline: 1.3928x; 1.3928x over previous
"""Trainium2 Bass kernel for nn_MACEConvolutionLayer.

Strategy (8 NeuronCores, no collectives):
  - Edges sharded by destination-node range (1250 nodes/core), sorted and
    packed into 10 windows of 128 nodes x 1024 edge slots per core. Messages
    are segment-summed into node windows via host-precomputed one-hot
    scatter matmuls on the tensor engine.
  - Per-edge bilinear (radial features x embedded source scalars) and the
    per-node equivariant tensor products use a monomial scheme computed
    directly in transposed [uv, sample] layout: replicated factor tiles
    (built by DMA through a DRAM scratch roundtrip) are multiplied
    elementwise on DVE/GpSimd, and the tensor engine contracts the monomial
    chunks against packed combined weight matrices (Clebsch-Gordan x TP
    weights with channel mixing/combination folded in). This avoids all
    PE-transposes and PSUM evacuation copies of the previous scheme.
  - Output q|msg columns are interleaved per kappa-component so each
    (block, chunk) usually emits one contiguous column run.

Feature layout on device is kappa-major: col(l, i, u) = LOFF[l] + i*32 + u.
"""
import sys, os

sys.path.insert(0, '/opt/trn_rl_repo')

import numpy as np
import ml_dtypes

MUL = 32
DIMS = (1, 3, 5)
HID = 288
N_NODES = 10000
N_EDGES = 64000
RHID = 64
SQM = float(np.sqrt(MUL))
LOFF = [0, 32, 128]
SOFF = [0, 1, 4]
PATHS_FULL = [(0,0,0),(0,1,1),(0,2,2),(1,0,1),(1,1,0),(1,1,2),(1,2,1),(2,0,2),(2,1,1),(2,2,0),(2,2,2)]
O2_UVW = [(0,1,1),(0,2,2),(1,2,1)]
O2_UVU = [(0,0,0),(1,1,0),(1,1,2),(2,2,0),(2,2,2)]

N_CORES = 8
NODES_PER_CORE = 1250
WIN = 128
N_WIN = 10
ESLOT = 1024
E_PAD = N_WIN * ESLOT   # 10240
ET_PER_WIN = ESLOT // 128  # 8
BF = ml_dtypes.bfloat16

NCOMP = 9  # number of (l, i) components
MAX_JRUN = 3   # max J-run length per product op


def comp_ord(l, i):
    return LOFF[l] // 32 + i


COMP_L = [0, 1, 1, 1, 2, 2, 2, 2, 2]  # l of each component ordinal


def cg_np():
    s2, s3, s5, s6 = map(np.sqrt, (2.0, 3.0, 5.0, 6.0))
    B = np.zeros((5, 3, 3))
    B[0, 0, 1] = B[0, 1, 0] = 1 / s2
    B[1, 1, 2] = B[1, 2, 1] = 1 / s2
    B[2] = np.diag([-1.0, -1.0, 2.0]) / s6
    B[3, 0, 2] = B[3, 2, 0] = 1 / s2
    B[4] = np.diag([1.0, -1.0, 0.0]) / s2
    C = {}
    C[(0, 0, 0)] = np.ones((1, 1, 1))
    C[(0, 1, 1)] = (np.eye(3) / s3)[None]
    C[(1, 0, 1)] = np.transpose(C[(0, 1, 1)], (1, 0, 2))
    C[(0, 2, 2)] = (np.eye(5) / s5)[None]
    C[(2, 0, 2)] = np.transpose(C[(0, 2, 2)], (1, 0, 2))
    C[(1, 1, 0)] = (np.eye(3) / s3)[:, :, None]
    C[(1, 1, 2)] = np.transpose(B, (1, 2, 0)) / s5
    C[(1, 2, 1)] = np.transpose(B, (1, 0, 2)) / s5
    C[(2, 1, 1)] = B / s5
    C[(2, 2, 0)] = (np.eye(5) / s5)[:, :, None]
    T = np.einsum('aij,bjk,cki->abc', B, B, B)
    C[(2, 2, 2)] = T / np.linalg.norm(T)
    return C


CG = cg_np()
PATH_LIST_O2 = O2_UVW + O2_UVU


def support_pairs(path_ijk):
    d = {}
    for pi, (li, lj, lk) in enumerate(path_ijk):
        C = CG[(li, lj, lk)]
        for iloc in range(DIMS[li]):
            for jloc in range(DIMS[lj]):
                if np.any(np.abs(C[iloc, jloc, :]) > 1e-12):
                    d.setdefault(((li, iloc), (lj, jloc)), []).append((pi, iloc, jloc))
    return d


def build_mono_blocks_sym(path_ijk):
    d = support_pairs(path_ijk)
    blocks = {}
    for (I, J), lst in d.items():
        key = (min(I, J), max(I, J))
        swap = I > J
        for (pi, iloc, jloc) in lst:
            blocks.setdefault(key, []).append((pi, iloc, jloc, swap))
    return [(I, J, c) for (I, J), c in sorted(blocks.items())]


def build_mono_blocks(path_ijk):
    d = support_pairs(path_ijk)
    return [(I, J, [(pi, i, j, False) for (pi, i, j) in lst]) for (I, J), lst in sorted(d.items())]


def omega_for_block(path_ijk, weights, I, J, contribs, reg):
    """[1024 (u-major,v-fast), 576] interleaved outputs:
    col(g_out, reg, w) = g_out*64 + reg*32 + w."""
    Om = np.zeros((MUL * MUL, 2 * HID))
    for (pi, iloc, jloc, swap) in contribs:
        li, lj, lk = path_ijk[pi]
        W = weights[pi]
        C = CG[(li, lj, lk)]
        for kap in range(DIMS[lk]):
            c = C[iloc, jloc, kap]
            if abs(c) < 1e-12:
                continue
            gk = comp_ord(lk, kap)
            c0 = gk * 64 + reg * 32
            Wm = W if not swap else np.transpose(W, (1, 0, 2))
            Om[:, c0:c0 + 32] += c * Wm.reshape(MUL * MUL, MUL)
    return Om


# ---------------------------------------------------------------------------
# static plan
# ---------------------------------------------------------------------------

class Plan:
    pass


def _emissions(mask):
    """mask: [1024, 576] bool. Returns per kc: list of (c0, c1) col runs
    (gaptol 0 at 32-col-slot granularity, split at 512-wide)."""
    out = []
    for kc in range(8):
        sub = mask[kc * 128:(kc + 1) * 128]
        slots = [s for s in range(18) if np.any(sub[:, s * 32:(s + 1) * 32])]
        runs = []
        for s in slots:
            if runs and s == runs[-1][1]:
                runs[-1][1] = s + 1
            else:
                runs.append([s, s + 1])
        emis = []
        for (a, b) in runs:
            while (b - a) * 32 > 512:
                emis.append((a * 32, a * 32 + 512))
                a += 16
            emis.append((a * 32, b * 32))
        out.append(emis)
    return out


def build_plan():
    p = Plan()
    aa_blocks = build_mono_blocks_sym(PATHS_FULL + PATH_LIST_O2)
    qa_blocks = build_mono_blocks(PATHS_FULL)
    n3a = len(PATHS_FULL)
    ones_a = [np.ones((MUL, MUL, MUL)) for _ in PATHS_FULL]
    ones_o2 = [np.ones((MUL, MUL, MUL)) for _ in PATH_LIST_O2]

    p.aa = []
    for (I, J, contribs) in aa_blocks:
        cq = [(pi, i, j, s) for (pi, i, j, s) in contribs if pi < n3a]
        cm = [(pi - n3a, i, j, s) for (pi, i, j, s) in contribs if pi >= n3a]
        mask = np.zeros((1024, 576), bool)
        if cq:
            mask |= omega_for_block(PATHS_FULL, ones_a, I, J, cq, 0) != 0
        if cm:
            mask |= omega_for_block(PATH_LIST_O2, ones_o2, I, J, cm, 1) != 0
        p.aa.append((I, J, cq, cm, _emissions(mask)))
    p.qa = []
    for (I, J, contribs) in qa_blocks:
        mask = omega_for_block(PATHS_FULL, ones_a, I, J, contribs, 1) != 0
        p.qa.append((I, J, contribs, _emissions(mask)))

    # omega column offsets
    off = 0
    p.aa_emi = []
    for (I, J, cq, cm, em) in p.aa:
        bk = []
        for kc in range(8):
            lst = []
            for (c0, c1) in em[kc]:
                lst.append((c0, c1, off))
                off += c1 - c0
            bk.append(lst)
        p.aa_emi.append(bk)
    p.qa_emi = []
    for (I, J, contribs, em) in p.qa:
        bk = []
        for kc in range(8):
            lst = []
            for (c0, c1) in em[kc]:
                lst.append((c0, c1, off))
                off += c1 - c0
            bk.append(lst)
        p.qa_emi.append(bk)
    p.totc = off
    p.n_emi = sum(len(l) for bk in p.aa_emi + p.qa_emi for l in bk)

    # J-run groups for product ops: consecutive blocks with same I and
    # consecutive J ordinals, capped at MAX_JRUN
    def groups(blocks):
        gs = []
        for bi, blk in enumerate(blocks):
            I, J = blk[0], blk[1]
            gI = comp_ord(*I); gJ = comp_ord(*J)
            if (gs and gs[-1][0] == gI and gs[-1][1] + gs[-1][2] == gJ
                    and gs[-1][2] < MAX_JRUN):
                gs[-1][2] += 1
            else:
                gs.append([gI, gJ, 1, bi])
        return [(gI, gJ, n, b0) for (gI, gJ, n, b0) in gs]

    p.aa_groups = groups(p.aa)
    p.qa_groups = groups(p.qa)
    return p


def pack_omega(plan, Wfold):
    W3a = Wfold['o3a_w']; Wo2 = Wfold['o2_w']; W3b = Wfold['o3b_w']
    om = np.zeros((128, plan.totc), np.float32)
    for bi, (I, J, cq, cm, em) in enumerate(plan.aa):
        Om = np.zeros((MUL * MUL, 2 * HID))
        if cq:
            Om += omega_for_block(PATHS_FULL, W3a, I, J, cq, 0)
        if cm:
            Om += omega_for_block(PATH_LIST_O2, Wo2, I, J, cm, 1)
        for kc in range(8):
            for (c0, c1, off) in plan.aa_emi[bi][kc]:
                om[:, off:off + (c1 - c0)] = Om[kc * 128:(kc + 1) * 128, c0:c1]
    for bi, (I, J, contribs, em) in enumerate(plan.qa):
        Om = omega_for_block(PATHS_FULL, W3b, I, J, contribs, 1)
        for kc in range(8):
            for (c0, c1, off) in plan.qa_emi[bi][kc]:
                om[:, off:off + (c1 - c0)] = Om[kc * 128:(kc + 1) * 128, c0:c1]
    return om.astype(BF)


def fold_weights(inp):
    f8 = np.float64
    mix_w = inp['mix_w'].astype(f8); comb_w = inp['comb_w'].astype(f8)
    M = np.einsum('olux,olxw->oluw', mix_w, comb_w) / MUL
    W1eff = np.einsum('lux,lxw->luw', inp['lin_o1'].astype(f8), M[0]) / SQM
    o2_w = []
    for pp, (i, j, k) in enumerate(O2_UVW):
        o2_w.append(np.einsum('uvx,xw->uvw', inp['o2_uvw'][pp].astype(f8) / MUL, M[1][k]))
    for pp, (i, j, k) in enumerate(O2_UVU):
        o2_w.append(np.einsum('uv,uw->uvw', inp['o2_uvu'][pp].astype(f8), M[1][k]) / SQM)
    o3a_w = [inp['o3a_uvw'][pp].astype(f8) / MUL for pp in range(len(PATHS_FULL))]
    o3b_w = [np.einsum('uvx,xw->uvw', inp['o3b_uvw'][pp].astype(f8) / MUL, M[2][k])
             for pp, (i, j, k) in enumerate(PATHS_FULL)]
    aw = inp['a_w'].astype(f8).reshape(RHID, 3, MUL, MUL)
    ab = inp['a_b'].astype(f8).reshape(3, MUL, MUL)
    scale = np.array([1.0 / np.sqrt(d) for d in DIMS]) / SQM
    aw = aw * scale[None, :, None, None]
    ab = ab * scale[:, None, None]
    A2 = np.transpose(aw, (0, 2, 1, 3)).reshape(RHID * MUL, 3 * MUL)
    B2 = np.transpose(ab, (1, 0, 2)).reshape(MUL, 3 * MUL)
    # omc1: [32, 3*32]: per-l 32x32 order-1 linear (same for all i of that l)
    omc1 = np.zeros((32, 96))
    for l in range(3):
        omc1[:, l * 32:(l + 1) * 32] = W1eff[l]
    return dict(
        o3a_w=o3a_w, o2_w=o2_w, o3b_w=o3b_w,
        omc1=omc1, omself=inp['self_w'].astype(f8) / SQM,
        emb=inp['emb_w'].astype(f8) / SQM,
        A2=A2, B2=B2,
        r_w1=inp['r_w1'].astype(np.float32), r_b1=inp['r_b1'].astype(np.float32),
        r_w2=inp['r_w2'].astype(np.float32), r_b2=inp['r_b2'].astype(np.float32),
        r_w3=inp['r_w3'].astype(np.float32), r_b3=inp['r_b3'].astype(np.float32),
    )


def pack_edges(inp):
    src = np.asarray(inp['edge_index'][0]).astype(np.int64)
    dst = np.asarray(inp['edge_index'][1]).astype(np.int64)
    sh = np.asarray(inp['edge_sh'], dtype=np.float32)
    rad = np.asarray(inp['edge_radial_embedding'], dtype=np.float32)
    attr = np.asarray(inp['edge_attr'], dtype=np.float32)
    nf = np.asarray(inp['node_features'], dtype=np.float32)
    cnt = np.bincount(dst, minlength=N_NODES).astype(np.float32)
    rec_all = 1.0 / np.maximum(cnt, 1.0)
    order = np.argsort(dst, kind='stable')
    dst_s = dst[order]
    cores = []
    for c in range(N_CORES):
        lo = c * NODES_PER_CORE
        rinT = np.zeros((24, E_PAD), np.float32)
        nfsT = np.zeros((MUL, E_PAD), np.float32)
        sh9 = np.zeros((E_PAD, 9), np.float32)
        S = np.zeros((E_PAD, 128), BF)
        for w in range(N_WIN):
            nlo = lo + w * WIN
            nhi = min(lo + (w + 1) * WIN, lo + NODES_PER_CORE)
            a = np.searchsorted(dst_s, nlo); b = np.searchsorted(dst_s, nhi)
            idx = order[a:b]
            n = b - a
            assert n <= ESLOT, f"window overflow {n}"
            s = w * ESLOT
            rinT[:8, s:s + n] = rad[idx].T
            rinT[8:, s:s + n] = attr[idx].T
            nfsT[:, s:s + n] = nf[src[idx]].T
            sh9[s:s + n, :] = sh[idx]
            S[s + np.arange(n), (dst[idx] - nlo)] = BF(1.0)
        nfT = np.zeros((MUL, N_WIN * WIN), BF)
        nfT[:, :NODES_PER_CORE] = nf[lo:lo + NODES_PER_CORE].T.astype(BF)
        rec = np.ones((N_WIN * WIN, 1), np.float32)
        rec[:NODES_PER_CORE, 0] = rec_all[lo:lo + NODES_PER_CORE]
        cores.append(dict(rinT=rinT, nfsT=nfsT, sh9=sh9, S=S, nfT=nfT, rec=rec))
    return cores


def ref_from_kap(x_kap):
    out = np.empty_like(x_kap)
    for l, d in enumerate(DIMS):
        blk = x_kap[:, LOFF[l]:LOFF[l] + 32 * d].reshape(-1, d, 32)
        out[:, LOFF[l]:LOFF[l] + 32 * d] = np.transpose(blk, (0, 2, 1)).reshape(-1, 32 * d)
    return out


# ---------------------------------------------------------------------------
# device kernel
# ---------------------------------------------------------------------------

_NC_CACHE = {}
LAST_RESULT = None

# fraction of product work sent to gpsimd (tuned from profiles)
GP_ELEM_NS = 2.05e-3   # us per elem per partition-row... (us per free-elem)
VE_ELEM_NS = 0.52e-3
GP_OP_OH = 0.25
VE_OP_OH = 0.08


def build_nc(plan):
    import concourse.bass as bass
    import concourse.bacc as bacc
    import concourse.mybir as mybir
    import concourse.tile as tile

    f32 = mybir.dt.float32
    bf16 = mybir.dt.bfloat16
    AL = mybir.AluOpType
    AF = mybir.ActivationFunctionType

    nc = bacc.Bacc(None)
    P = 128

    # ---- dram parameters
    rinT_d = nc.declare_dram_parameter("rinT", [24, E_PAD], f32, isOutput=False)
    nfsT_d = nc.declare_dram_parameter("nfsT", [32, E_PAD], f32, isOutput=False)
    sh9_d = nc.declare_dram_parameter("sh9", [E_PAD, 9], f32, isOutput=False)
    S_d = nc.declare_dram_parameter("S", [E_PAD, 128], bf16, isOutput=False)
    nfT_d = nc.declare_dram_parameter("nfT", [32, N_WIN * WIN], bf16, isOutput=False)
    rec_d = nc.declare_dram_parameter("rec", [N_WIN * WIN, 1], f32, isOutput=False)
    omega_d = nc.declare_dram_parameter("omega", [P, plan.totc], bf16, isOutput=False)
    a2_d = nc.declare_dram_parameter("a2", [P, 16 * 96], bf16, isOutput=False)
    b2_d = nc.declare_dram_parameter("b2", [32, 96], bf16, isOutput=False)
    omc1_d = nc.declare_dram_parameter("omc1", [32, 96], bf16, isOutput=False)
    omself_d = nc.declare_dram_parameter("omself", [32, 32], bf16, isOutput=False)
    rw1_d = nc.declare_dram_parameter("rw1", [24, 64], f32, isOutput=False)
    rw2_d = nc.declare_dram_parameter("rw2", [64, 64], f32, isOutput=False)
    rw3_d = nc.declare_dram_parameter("rw3", [64, 64], f32, isOutput=False)
    rb1_d = nc.declare_dram_parameter("rb1", [64, 1], f32, isOutput=False)
    rb2_d = nc.declare_dram_parameter("rb2", [64, 1], f32, isOutput=False)
    emb_d = nc.declare_dram_parameter("emb", [32, 32], f32, isOutput=False)
    identb_d = nc.declare_dram_parameter("identb", [P, P], bf16, isOutput=False)
    zer_d = nc.declare_dram_parameter("zer", [1, P], bf16, isOutput=False)
    zer2_d = nc.declare_dram_parameter("zer2", [1, 2 * HID], bf16, isOutput=False)
    out_d = nc.declare_dram_parameter("out", [N_WIN * WIN, HID], f32, isOutput=True)

    # engine schedule for product ops: greedy balance vector vs gpsimd
    def make_sched():
        ops = []   # (kind, idx, width_elems)
        for gi, (gI, gJ, nJ, b0) in enumerate(plan.aa_groups):
            ops.append(('aa', gi, nJ * 1024))
        for gi, (gI, gJ, nJ, b0) in enumerate(plan.qa_groups):
            ops.append(('qa', gi, nJ * 1024))
        for t in range(ET_PER_WIN):
            ops.append(('edge', t, 2048))
        v_t, g_t = 1.5, 0.0   # vector starts with msgs/evac budget
        sched = {}
        for (kind, idx, wdt) in ops:
            vc = wdt * VE_ELEM_NS + VE_OP_OH
            gc = wdt * GP_ELEM_NS + GP_OP_OH
            if g_t + gc < v_t + vc:
                sched[(kind, idx)] = 'gpsimd'; g_t += gc
            else:
                sched[(kind, idx)] = 'vector'; v_t += vc
        return sched

    sched = make_sched()

    from contextlib import ExitStack
    with tile.TileContext(nc) as tc, ExitStack() as es:
        cst = es.enter_context(tc.tile_pool(name="cst", bufs=1))
        sb2 = es.enter_context(tc.tile_pool(name="sb2", bufs=2))
        sb3 = es.enter_context(tc.tile_pool(name="sb3", bufs=3))
        uu_pool = es.enter_context(tc.tile_pool(name="uu", bufs=1))
        pt_pool = es.enter_context(tc.tile_pool(name="pt", bufs=2))
        ed_pool = es.enter_context(tc.tile_pool(name="ed", bufs=2))
        dr = es.enter_context(tc.tile_pool(name="dr", bufs=2, space="DRAM"))
        ps_wps = es.enter_context(tc.tile_pool(name="pswps", bufs=2, space="PSUM"))
        ps_qm = es.enter_context(tc.tile_pool(name="psqm", bufs=1, space="PSUM"))
        ps_tp = es.enter_context(tc.tile_pool(name="pstp", bufs=1, space="PSUM"))
        ps_mlp = es.enter_context(tc.tile_pool(name="psmlp", bufs=1, space="PSUM"))
        ps_mx = es.enter_context(tc.tile_pool(name="psmx", bufs=2, space="PSUM"))

        # ---- constants
        omega = cst.tile([P, plan.totc], bf16)
        nc.sync.dma_start(out=omega[:], in_=omega_d[:])
        a2 = cst.tile([P, 16 * 96], bf16)
        nc.sync.dma_start(out=a2[:], in_=a2_d[:])
        b2 = cst.tile([32, 96], bf16); nc.sync.dma_start(out=b2[:], in_=b2_d[:])
        omc1 = cst.tile([32, 96], bf16); nc.sync.dma_start(out=omc1[:], in_=omc1_d[:])
        omself = cst.tile([32, 32], bf16); nc.sync.dma_start(out=omself[:], in_=omself_d[:])
        rw1 = cst.tile([24, 64], f32); nc.sync.dma_start(out=rw1[:], in_=rw1_d[:])
        rw2 = cst.tile([64, 64], f32); nc.sync.dma_start(out=rw2[:], in_=rw2_d[:])
        rw3 = cst.tile([64, 64], f32); nc.sync.dma_start(out=rw3[:], in_=rw3_d[:])
        rb1 = cst.tile([64, 1], f32); nc.sync.dma_start(out=rb1[:], in_=rb1_d[:])
        rb2 = cst.tile([64, 1], f32); nc.sync.dma_start(out=rb2[:], in_=rb2_d[:])
        emb = cst.tile([32, 32], f32); nc.sync.dma_start(out=emb[:], in_=emb_d[:])
        identb = cst.tile([P, P], bf16); nc.sync.dma_start(out=identb[:], in_=identb_d[:])
        zer = cst.tile([1, P], bf16); nc.sync.dma_start(out=zer[:], in_=zer_d[:])
        zer2 = cst.tile([1, 2 * HID], bf16); nc.sync.dma_start(out=zer2[:], in_=zer2_d[:])
        nfT = cst.tile([32, N_WIN * WIN], bf16)
        nc.sync.dma_start(out=nfT[:], in_=nfT_d[:])

        def transpose3_to_dram(x_bf, tag):
            """x_bf [128, 288] bf16 -> DRAM scratch [384, 128] (f-major rows)."""
            tp = ps_tp.tile([P, 384], bf16, space="PSUM", tag="tp")
            nc.tensor.transpose(out=tp[:, 0:P], in_=x_bf[:, 0:P], identity=identb[:])
            nc.tensor.transpose(out=tp[:, P:2 * P], in_=x_bf[:, P:2 * P], identity=identb[:])
            nc.tensor.transpose(out=tp[0:32, 2 * P:3 * P], in_=x_bf[:, 2 * P:HID], identity=identb[:])
            xt = sb2.tile([P, 384], bf16, tag=tag + "sb")
            nc.scalar.copy(out=xt[:, 0:2 * P], in_=tp[:, 0:2 * P])
            nc.scalar.copy(out=xt[0:32, 2 * P:3 * P], in_=tp[0:32, 2 * P:3 * P])
            xd = dr.tile([384, P], bf16, tag=tag + "d")
            nc.sync.dma_start(
                out=xd[:].rearrange("(c p) n -> p c n", p=P),
                in_=xt[:].rearrange("p (c n) -> p c n", c=3))
            return xd

        def replicate_uu(xd, uu_tile):
            """uu[32*u4+v, (g,kc,n)] = xd[32g + 4kc + u4, n]"""
            sub = xd[:].rearrange("(g k r) n -> g k r n", k=8, r=4)
            for u4 in range(4):
                src = sub[0:NCOMP, :, u4, :].rearrange("g k n -> (g k) n")
                nc.scalar.dma_start(
                    out=uu_tile[32 * u4:32 * (u4 + 1), :].rearrange("v (gk n) -> v gk n", n=P),
                    in_=src[None, :, :].broadcast_to([32, NCOMP * 8, P]))

        def replicate_v8(xd, v8_tile):
            """v8[32*b+v, (g,n)] = xd[32g + v, n]"""
            svb = xd[:].rearrange("(g v) n -> v g n", v=32)
            for b in range(4):
                nc.scalar.dma_start(
                    out=v8_tile[32 * b:32 * (b + 1), :].rearrange("v (g n) -> v g n", n=P),
                    in_=svb[:, 0:NCOMP, :])

        def products_and_emissions(groups, blocks, emi, uu, v8, qm, kind):
            for gi, (gI, gJ, nJ, b0) in enumerate(groups):
                w = nJ * 1024
                PT = pt_pool.tile([P, MAX_JRUN * 1024], bf16, tag="PT")
                eng = nc.gpsimd if sched[(kind, gi)] == 'gpsimd' else nc.vector
                eng.tensor_tensor(
                    out=PT[:, :w].rearrange("p (j k n) -> p j k n", k=8, n=P),
                    in0=uu[:, gI * 1024:(gI + 1) * 1024]
                        .rearrange("p (k n) -> p k n", n=P)[:, None, :, :]
                        .broadcast_to([P, nJ, 8, P]),
                    in1=v8[:, gJ * P:(gJ + nJ) * P]
                        .rearrange("p (j n) -> p j n", n=P)[:, :, None, :]
                        .broadcast_to([P, nJ, 8, P]),
                    op=AL.mult)
                for jl in range(nJ):
                    bi = b0 + jl
                    for kc in range(8):
                        for (c0, c1, off) in emi[bi][kc]:
                            nc.tensor.matmul(out=qm[:, c0:c1],
                                             lhsT=PT[:, jl * 1024 + kc * P: jl * 1024 + (kc + 1) * P],
                                             rhs=omega[:, off:off + (c1 - c0)],
                                             start=False, stop=False,
                                             skip_group_check=True)

        # ---------------- main loop ----------------
        for w in range(N_WIN):
            e0 = w * ESLOT
            # ---- edge phase: radial MLP + embedding, in 2 halves of 512
            rfT = sb2.tile([64, ESLOT], bf16, tag="rfT")
            hT = sb2.tile([32, ESLOT], bf16, tag="hT")
            for h in range(2):
                s = e0 + h * 512
                rin_h = sb2.tile([24, 512], f32, tag="rin")
                nc.sync.dma_start(out=rin_h[:], in_=rinT_d[:, s:s + 512])
                nfs_h = sb2.tile([32, 512], f32, tag="nfs")
                nc.sync.dma_start(out=nfs_h[:], in_=nfsT_d[:, s:s + 512])
                l1p = ps_mlp.tile([64, 512], f32, space="PSUM", tag="mlp")
                nc.tensor.matmul(out=l1p[:], lhsT=rw1[:], rhs=rin_h[:], start=True, stop=True)
                f1 = sb2.tile([64, 512], f32, tag="f")
                nc.scalar.activation(out=f1[:], in_=l1p[:], func=AF.Silu, bias=rb1[:], scale=1.0)
                l2p = ps_mlp.tile([64, 512], f32, space="PSUM", tag="mlp")
                nc.tensor.matmul(out=l2p[:], lhsT=rw2[:], rhs=f1[:], start=True, stop=True)
                f2 = sb2.tile([64, 512], f32, tag="f")
                nc.scalar.activation(out=f2[:], in_=l2p[:], func=AF.Silu, bias=rb2[:], scale=1.0)
                rfp = ps_mlp.tile([64, 512], f32, space="PSUM", tag="mlp")
                nc.tensor.matmul(out=rfp[:], lhsT=rw3[:], rhs=f2[:], start=True, stop=True)
                nc.vector.tensor_copy(out=rfT[:, h * 512:(h + 1) * 512], in_=rfp[:])
                hp = ps_mlp.tile([32, 512], f32, space="PSUM", tag="mlp")
                nc.tensor.matmul(out=hp[:], lhsT=emb[:], rhs=nfs_h[:], start=True, stop=True)
                nc.scalar.copy(out=hT[:, h * 512:(h + 1) * 512], in_=hp[:])
            # rf to DRAM for replication
            rfd = dr.tile([64, ESLOT], bf16, tag="rfd")
            nc.sync.dma_start(out=rfd[:], in_=rfT[:])
            # V_h: hT replicated mod-32 across partitions
            vh = sb2.tile([P, ESLOT], bf16, tag="vh")
            for b in range(4):
                nc.sync.dma_start(out=vh[32 * b:32 * (b + 1), :], in_=hT[:])

            wps = ps_wps.tile([P, HID], f32, space="PSUM", tag="wps")
            rfsub = rfd[:].rearrange("(c r) e -> c r e", r=4)
            for t in range(ET_PER_WIN):
                et = e0 + t * P
                # UU_rf: [128, (c16, e128)] = rfT[4c + p//32, e] via DRAM
                uurf = ed_pool.tile([P, 2048], bf16, tag="uurf")
                for u4 in range(4):
                    src = rfsub[:, u4, t * P:(t + 1) * P]
                    nc.sync.dma_start(
                        out=uurf[32 * u4:32 * (u4 + 1), :].rearrange("v (c e) -> v c e", e=P),
                        in_=src[None, :, :].broadcast_to([32, 16, P]))
                mT = ed_pool.tile([P, 2048], bf16, tag="mT")
                eng = nc.gpsimd if sched[('edge', t)] == 'gpsimd' else nc.vector
                eng.tensor_tensor(
                    out=mT[:].rearrange("p (c e) -> p c e", e=P),
                    in0=uurf[:].rearrange("p (c e) -> p c e", e=P),
                    in1=vh[:, t * P:(t + 1) * P][:, None, :].broadcast_to([P, 16, P]),
                    op=AL.mult)
                # mixed = m @ A2 + h @ B2
                mxp = ps_mx.tile([P, 96], f32, space="PSUM", tag="mx")
                for c in range(16):
                    nc.tensor.matmul(out=mxp[:], lhsT=mT[:, c * P:(c + 1) * P],
                                     rhs=a2[:, c * 96:(c + 1) * 96],
                                     start=(c == 0), stop=False)
                nc.tensor.matmul(out=mxp[:], lhsT=hT[:, t * P:(t + 1) * P], rhs=b2[:],
                                 start=False, stop=True)
                # messages msgs[e, 32 g + u] = sh[e, comp(g)] * mixed[e, l*32+u]
                sh_t = sb3.tile([P, 9], f32, tag="sht")
                nc.sync.dma_start(out=sh_t[:], in_=sh9_d[et:et + P, :])
                msgs = sb3.tile([P, HID], bf16, tag="msgs")
                for l, d in enumerate(DIMS):
                    nc.vector.tensor_tensor(
                        out=msgs[:, LOFF[l]:LOFF[l] + 32 * d].rearrange("p (i u) -> p i u", u=32),
                        in0=sh_t[:, SOFF[l]:SOFF[l] + d][:, :, None].broadcast_to([P, d, 32]),
                        in1=mxp[:, l * 32:(l + 1) * 32][:, None, :].broadcast_to([P, d, 32]),
                        op=AL.mult)
                S_t = sb3.tile([P, P], bf16, tag="St")
                nc.sync.dma_start(out=S_t[:], in_=S_d[et:et + P, :])
                nc.tensor.matmul(out=wps[:], lhsT=S_t[:], rhs=msgs[:],
                                 start=(t == 0), stop=(t == ET_PER_WIN - 1))

            # ---- node phase
            rec_t = sb2.tile([P, 1], f32, tag="rec")
            nc.sync.dma_start(out=rec_t[:], in_=rec_d[w * P:(w + 1) * P, :])
            a_bf = sb2.tile([P, HID], bf16, tag="abf")
            nc.vector.tensor_scalar_mul(out=a_bf[:], in0=wps[:], scalar1=rec_t[:])
            atd = transpose3_to_dram(a_bf, "at")
            uu = uu_pool.tile([P, NCOMP * 1024], bf16, tag="uu")
            replicate_uu(atd, uu)
            v8 = sb2.tile([P, NCOMP * P], bf16, tag="v8")
            replicate_v8(atd, v8)

            qm = ps_qm.tile([P, 2 * HID], f32, space="PSUM", tag="qm")
            nc.tensor.matmul(out=qm[:, 0:512], lhsT=zer[:], rhs=zer2[:, 0:512], start=True, stop=False, skip_group_check=True)
            nc.tensor.matmul(out=qm[:, 512:576], lhsT=zer[:], rhs=zer2[:, 512:576], start=True, stop=False, skip_group_check=True)

            products_and_emissions(plan.aa_groups, plan.aa, plan.aa_emi, uu, v8, qm, 'aa')

            # q evacuation (strided: even 32-col slots), then q replication
            q_bf = sb2.tile([P, HID], bf16, tag="qbf")
            nc.vector.tensor_copy(
                out=q_bf[:].rearrange("p (g c) -> p g c", c=32),
                in_=qm[:].rearrange("p (g t c) -> p g t c", t=2, c=32)[:, :, 0, :])
            qtd = transpose3_to_dram(q_bf, "qt")
            uuq = uu_pool.tile([P, NCOMP * 1024], bf16, tag="uuq")
            replicate_uu(qtd, uuq)

            products_and_emissions(plan.qa_groups, plan.qa, plan.qa_emi, uuq, v8, qm, 'qa')

            # c1: per-component order-1 linear into msg slots (lhsT = V8 rows 0:32)
            for g in range(NCOMP):
                l = COMP_L[g]
                nc.tensor.matmul(out=qm[:, g * 64 + 32:g * 64 + 64],
                                 lhsT=v8[0:32, g * P:(g + 1) * P],
                                 rhs=omc1[:, l * 32:(l + 1) * 32],
                                 start=False, stop=False, skip_group_check=True)
            # self connection: scalar block = component 0 msg slot
            nc.tensor.matmul(out=qm[:, 32:64], lhsT=nfT[:, w * P:(w + 1) * P],
                             rhs=omself[:], start=False, stop=True,
                             skip_group_check=True)
            # evacuate msg slots (odd 32-col slots)
            out_sb = sb2.tile([P, HID], f32, tag="outsb")
            nc.scalar.copy(
                out=out_sb[:].rearrange("p (g c) -> p g c", c=32),
                in_=qm[:].rearrange("p (g t c) -> p g t c", t=2, c=32)[:, :, 1, :])
            nc.sync.dma_start(out=out_d[w * P:(w + 1) * P, :], in_=out_sb[:])

    nc.finalize()
    return nc


def _get_nc(plan):
    if 'nc' not in _NC_CACHE:
        _NC_CACHE['nc'] = build_nc(plan)
    return _NC_CACHE['nc']


def kernel(**inputs):
    global LAST_RESULT
    from concourse.bass_utils import run_bass_kernel_spmd

    inp = {k: np.asarray(v) for k, v in inputs.items()}
    plan = build_plan()
    W = fold_weights(inp)
    om = pack_omega(plan, W)

    A2 = W['A2'].astype(np.float32)
    a2p = np.zeros((128, 16 * 96), np.float32)
    for c in range(16):
        a2p[:, c * 96:(c + 1) * 96] = A2[c * 128:(c + 1) * 128, :]
    # fold r_b3 into B2 (rf = f2 @ rw3; +b3 contribution is linear in h)
    B2 = W['B2'].astype(np.float64).copy()
    b3 = inp['r_b3'].astype(np.float64)
    for u in range(32):
        B2[u, :] += b3 @ A2[np.arange(RHID) * 32 + u, :].astype(np.float64)

    identb = np.eye(128, dtype=np.float32).astype(BF)

    shared = dict(
        omega=om,
        a2=a2p.astype(BF), b2=B2.astype(np.float32).astype(BF),
        omc1=W['omc1'].astype(np.float32).astype(BF),
        omself=W['omself'].astype(np.float32).astype(BF),
        rw1=W['r_w1'], rw2=W['r_w2'], rw3=W['r_w3'],
        rb1=W['r_b1'].reshape(64, 1), rb2=W['r_b2'].reshape(64, 1),
        emb=W['emb'].astype(np.float32),
        identb=identb,
        zer=np.zeros((1, 128), BF), zer2=np.zeros((1, 2 * HID), BF),
    )
    cores = pack_edges(inp)
    in_maps = []
    for c in range(N_CORES):
        m = dict(shared)
        m.update(rinT=cores[c]['rinT'], nfsT=cores[c]['nfsT'],
                 sh9=cores[c]['sh9'], S=cores[c]['S'], nfT=cores[c]['nfT'],
                 rec=cores[c]['rec'])
        in_maps.append(m)

    nc = _get_nc(plan)
    res = run_bass_kernel_spmd(nc, in_maps, core_ids=list(range(N_CORES)))
    LAST_RESULT = res
    outs = [res.results[c]['out'][:NODES_PER_CORE] for c in range(N_CORES)]
    out_kap = np.concatenate(outs, axis=0).astype(np.float32)
    return ref_from_kap(out_kap)


if __name__ == "__main__":
    plan = build_plan()
    print(f"aa blocks: {len(plan.aa)}  qa blocks: {len(plan.qa)}")
    print(f"aa groups: {len(plan.aa_groups)}  qa groups: {len(plan.qa_groups)}")
    print(f"omega cols: {plan.totc}  ({plan.totc * 128 * 2 / 1e6:.1f} MB bf16)")
    print(f"emissions per node-tile: {plan.n_emi}")


# revision 11
# speedup vs baseline: 1.3992x; 1.0046x over previous
"""Trainium2 Bass kernel for nn_MACEConvolutionLayer.

Strategy (8 NeuronCores, no collectives):
  - Edges sharded by destination-node range (1250 nodes/core), sorted and
    packed into 10 windows of 128 nodes x 1024 edge slots per core. Messages
    are segment-summed into node windows via host-precomputed one-hot
    scatter matmuls on the tensor engine.
  - Per-edge bilinear (radial features x embedded source scalars) and the
    per-node equivariant tensor products use a monomial scheme computed
    directly in transposed [uv, sample] layout: replicated factor tiles
    (built by DMA through a DRAM scratch roundtrip) are multiplied
    elementwise on DVE/GpSimd, and the tensor engine contracts the monomial
    chunks against packed combined weight matrices (Clebsch-Gordan x TP
    weights with channel mixing/combination folded in). This avoids all
    PE-transposes and PSUM evacuation copies of the previous scheme.
  - Output q|msg columns are interleaved per kappa-component so each
    (block, chunk) usually emits one contiguous column run.

Feature layout on device is kappa-major: col(l, i, u) = LOFF[l] + i*32 + u.
"""
import sys, os

sys.path.insert(0, '/opt/trn_rl_repo')

import numpy as np
import ml_dtypes

MUL = 32
DIMS = (1, 3, 5)
HID = 288
N_NODES = 10000
N_EDGES = 64000
RHID = 64
SQM = float(np.sqrt(MUL))
LOFF = [0, 32, 128]
SOFF = [0, 1, 4]
PATHS_FULL = [(0,0,0),(0,1,1),(0,2,2),(1,0,1),(1,1,0),(1,1,2),(1,2,1),(2,0,2),(2,1,1),(2,2,0),(2,2,2)]
O2_UVW = [(0,1,1),(0,2,2),(1,2,1)]
O2_UVU = [(0,0,0),(1,1,0),(1,1,2),(2,2,0),(2,2,2)]

N_CORES = 8
NODES_PER_CORE = 1250
WIN = 128
N_WIN = 10
ESLOT = 1024
E_PAD = N_WIN * ESLOT   # 10240
ET_PER_WIN = ESLOT // 128  # 8
BF = ml_dtypes.bfloat16

NCOMP = 9  # number of (l, i) components
MAX_JRUN = 3   # max J-run length per product op


def comp_ord(l, i):
    return LOFF[l] // 32 + i


COMP_L = [0, 1, 1, 1, 2, 2, 2, 2, 2]  # l of each component ordinal


def cg_np():
    s2, s3, s5, s6 = map(np.sqrt, (2.0, 3.0, 5.0, 6.0))
    B = np.zeros((5, 3, 3))
    B[0, 0, 1] = B[0, 1, 0] = 1 / s2
    B[1, 1, 2] = B[1, 2, 1] = 1 / s2
    B[2] = np.diag([-1.0, -1.0, 2.0]) / s6
    B[3, 0, 2] = B[3, 2, 0] = 1 / s2
    B[4] = np.diag([1.0, -1.0, 0.0]) / s2
    C = {}
    C[(0, 0, 0)] = np.ones((1, 1, 1))
    C[(0, 1, 1)] = (np.eye(3) / s3)[None]
    C[(1, 0, 1)] = np.transpose(C[(0, 1, 1)], (1, 0, 2))
    C[(0, 2, 2)] = (np.eye(5) / s5)[None]
    C[(2, 0, 2)] = np.transpose(C[(0, 2, 2)], (1, 0, 2))
    C[(1, 1, 0)] = (np.eye(3) / s3)[:, :, None]
    C[(1, 1, 2)] = np.transpose(B, (1, 2, 0)) / s5
    C[(1, 2, 1)] = np.transpose(B, (1, 0, 2)) / s5
    C[(2, 1, 1)] = B / s5
    C[(2, 2, 0)] = (np.eye(5) / s5)[:, :, None]
    T = np.einsum('aij,bjk,cki->abc', B, B, B)
    C[(2, 2, 2)] = T / np.linalg.norm(T)
    return C


CG = cg_np()
PATH_LIST_O2 = O2_UVW + O2_UVU


def support_pairs(path_ijk):
    d = {}
    for pi, (li, lj, lk) in enumerate(path_ijk):
        C = CG[(li, lj, lk)]
        for iloc in range(DIMS[li]):
            for jloc in range(DIMS[lj]):
                if np.any(np.abs(C[iloc, jloc, :]) > 1e-12):
                    d.setdefault(((li, iloc), (lj, jloc)), []).append((pi, iloc, jloc))
    return d


def build_mono_blocks_sym(path_ijk):
    d = support_pairs(path_ijk)
    blocks = {}
    for (I, J), lst in d.items():
        key = (min(I, J), max(I, J))
        swap = I > J
        for (pi, iloc, jloc) in lst:
            blocks.setdefault(key, []).append((pi, iloc, jloc, swap))
    return [(I, J, c) for (I, J), c in sorted(blocks.items())]


def build_mono_blocks(path_ijk):
    d = support_pairs(path_ijk)
    return [(I, J, [(pi, i, j, False) for (pi, i, j) in lst]) for (I, J), lst in sorted(d.items())]


def omega_for_block(path_ijk, weights, I, J, contribs, reg):
    """[1024 (u-major,v-fast), 576] interleaved outputs:
    col(g_out, reg, w) = g_out*64 + reg*32 + w."""
    Om = np.zeros((MUL * MUL, 2 * HID))
    for (pi, iloc, jloc, swap) in contribs:
        li, lj, lk = path_ijk[pi]
        W = weights[pi]
        C = CG[(li, lj, lk)]
        for kap in range(DIMS[lk]):
            c = C[iloc, jloc, kap]
            if abs(c) < 1e-12:
                continue
            gk = comp_ord(lk, kap)
            c0 = gk * 64 + reg * 32
            Wm = W if not swap else np.transpose(W, (1, 0, 2))
            Om[:, c0:c0 + 32] += c * Wm.reshape(MUL * MUL, MUL)
    return Om


# ---------------------------------------------------------------------------
# static plan
# ---------------------------------------------------------------------------

class Plan:
    pass


def _emissions(mask):
    """mask: [1024, 576] bool. Returns per kc: list of (c0, c1) col runs
    (gaptol 0 at 32-col-slot granularity, split at 512-wide)."""
    out = []
    for kc in range(8):
        sub = mask[kc * 128:(kc + 1) * 128]
        slots = [s for s in range(18) if np.any(sub[:, s * 32:(s + 1) * 32])]
        runs = []
        for s in slots:
            if runs and s == runs[-1][1]:
                runs[-1][1] = s + 1
            else:
                runs.append([s, s + 1])
        emis = []
        for (a, b) in runs:
            while (b - a) * 32 > 512:
                emis.append((a * 32, a * 32 + 512))
                a += 16
            emis.append((a * 32, b * 32))
        out.append(emis)
    return out


def build_plan():
    p = Plan()
    aa_blocks = build_mono_blocks_sym(PATHS_FULL + PATH_LIST_O2)
    qa_blocks = build_mono_blocks(PATHS_FULL)
    n3a = len(PATHS_FULL)
    ones_a = [np.ones((MUL, MUL, MUL)) for _ in PATHS_FULL]
    ones_o2 = [np.ones((MUL, MUL, MUL)) for _ in PATH_LIST_O2]

    p.aa = []
    for (I, J, contribs) in aa_blocks:
        cq = [(pi, i, j, s) for (pi, i, j, s) in contribs if pi < n3a]
        cm = [(pi - n3a, i, j, s) for (pi, i, j, s) in contribs if pi >= n3a]
        mask = np.zeros((1024, 576), bool)
        if cq:
            mask |= omega_for_block(PATHS_FULL, ones_a, I, J, cq, 0) != 0
        if cm:
            mask |= omega_for_block(PATH_LIST_O2, ones_o2, I, J, cm, 1) != 0
        p.aa.append((I, J, cq, cm, _emissions(mask)))
    p.qa = []
    for (I, J, contribs) in qa_blocks:
        mask = omega_for_block(PATHS_FULL, ones_a, I, J, contribs, 1) != 0
        p.qa.append((I, J, contribs, _emissions(mask)))

    # omega column offsets
    off = 0
    p.aa_emi = []
    for (I, J, cq, cm, em) in p.aa:
        bk = []
        for kc in range(8):
            lst = []
            for (c0, c1) in em[kc]:
                lst.append((c0, c1, off))
                off += c1 - c0
            bk.append(lst)
        p.aa_emi.append(bk)
    p.qa_emi = []
    for (I, J, contribs, em) in p.qa:
        bk = []
        for kc in range(8):
            lst = []
            for (c0, c1) in em[kc]:
                lst.append((c0, c1, off))
                off += c1 - c0
            bk.append(lst)
        p.qa_emi.append(bk)
    p.totc = off
    p.n_emi = sum(len(l) for bk in p.aa_emi + p.qa_emi for l in bk)

    # J-run groups for product ops: consecutive blocks with same I and
    # consecutive J ordinals, capped at MAX_JRUN
    def groups(blocks):
        gs = []
        for bi, blk in enumerate(blocks):
            I, J = blk[0], blk[1]
            gI = comp_ord(*I); gJ = comp_ord(*J)
            if (gs and gs[-1][0] == gI and gs[-1][1] + gs[-1][2] == gJ
                    and gs[-1][2] < MAX_JRUN):
                gs[-1][2] += 1
            else:
                gs.append([gI, gJ, 1, bi])
        return [(gI, gJ, n, b0) for (gI, gJ, n, b0) in gs]

    p.aa_groups = groups(p.aa)
    p.qa_groups = groups(p.qa)
    return p


def pack_omega(plan, Wfold):
    W3a = Wfold['o3a_w']; Wo2 = Wfold['o2_w']; W3b = Wfold['o3b_w']
    om = np.zeros((128, plan.totc), np.float32)
    for bi, (I, J, cq, cm, em) in enumerate(plan.aa):
        Om = np.zeros((MUL * MUL, 2 * HID))
        if cq:
            Om += omega_for_block(PATHS_FULL, W3a, I, J, cq, 0)
        if cm:
            Om += omega_for_block(PATH_LIST_O2, Wo2, I, J, cm, 1)
        for kc in range(8):
            for (c0, c1, off) in plan.aa_emi[bi][kc]:
                om[:, off:off + (c1 - c0)] = Om[kc * 128:(kc + 1) * 128, c0:c1]
    for bi, (I, J, contribs, em) in enumerate(plan.qa):
        Om = omega_for_block(PATHS_FULL, W3b, I, J, contribs, 1)
        for kc in range(8):
            for (c0, c1, off) in plan.qa_emi[bi][kc]:
                om[:, off:off + (c1 - c0)] = Om[kc * 128:(kc + 1) * 128, c0:c1]
    return om.astype(BF)


def fold_weights(inp):
    f8 = np.float64
    mix_w = inp['mix_w'].astype(f8); comb_w = inp['comb_w'].astype(f8)
    M = np.einsum('olux,olxw->oluw', mix_w, comb_w) / MUL
    W1eff = np.einsum('lux,lxw->luw', inp['lin_o1'].astype(f8), M[0]) / SQM
    o2_w = []
    for pp, (i, j, k) in enumerate(O2_UVW):
        o2_w.append(np.einsum('uvx,xw->uvw', inp['o2_uvw'][pp].astype(f8) / MUL, M[1][k]))
    for pp, (i, j, k) in enumerate(O2_UVU):
        o2_w.append(np.einsum('uv,uw->uvw', inp['o2_uvu'][pp].astype(f8), M[1][k]) / SQM)
    o3a_w = [inp['o3a_uvw'][pp].astype(f8) / MUL for pp in range(len(PATHS_FULL))]
    o3b_w = [np.einsum('uvx,xw->uvw', inp['o3b_uvw'][pp].astype(f8) / MUL, M[2][k])
             for pp, (i, j, k) in enumerate(PATHS_FULL)]
    aw = inp['a_w'].astype(f8).reshape(RHID, 3, MUL, MUL)
    ab = inp['a_b'].astype(f8).reshape(3, MUL, MUL)
    scale = np.array([1.0 / np.sqrt(d) for d in DIMS]) / SQM
    aw = aw * scale[None, :, None, None]
    ab = ab * scale[:, None, None]
    A2 = np.transpose(aw, (0, 2, 1, 3)).reshape(RHID * MUL, 3 * MUL)
    B2 = np.transpose(ab, (1, 0, 2)).reshape(MUL, 3 * MUL)
    # omc1: [32, 3*32]: per-l 32x32 order-1 linear (same for all i of that l)
    omc1 = np.zeros((32, 96))
    for l in range(3):
        omc1[:, l * 32:(l + 1) * 32] = W1eff[l]
    return dict(
        o3a_w=o3a_w, o2_w=o2_w, o3b_w=o3b_w,
        omc1=omc1, omself=inp['self_w'].astype(f8) / SQM,
        emb=inp['emb_w'].astype(f8) / SQM,
        A2=A2, B2=B2,
        r_w1=inp['r_w1'].astype(np.float32), r_b1=inp['r_b1'].astype(np.float32),
        r_w2=inp['r_w2'].astype(np.float32), r_b2=inp['r_b2'].astype(np.float32),
        r_w3=inp['r_w3'].astype(np.float32), r_b3=inp['r_b3'].astype(np.float32),
    )


def pack_edges(inp):
    src = np.asarray(inp['edge_index'][0]).astype(np.int64)
    dst = np.asarray(inp['edge_index'][1]).astype(np.int64)
    sh = np.asarray(inp['edge_sh'], dtype=np.float32)
    rad = np.asarray(inp['edge_radial_embedding'], dtype=np.float32)
    attr = np.asarray(inp['edge_attr'], dtype=np.float32)
    nf = np.asarray(inp['node_features'], dtype=np.float32)
    cnt = np.bincount(dst, minlength=N_NODES).astype(np.float32)
    rec_all = 1.0 / np.maximum(cnt, 1.0)
    order = np.argsort(dst, kind='stable')
    dst_s = dst[order]
    cores = []
    for c in range(N_CORES):
        lo = c * NODES_PER_CORE
        rinT = np.zeros((24, E_PAD), np.float32)
        nfsT = np.zeros((MUL, E_PAD), np.float32)
        sh9 = np.zeros((E_PAD, 9), np.float32)
        S = np.zeros((E_PAD, 128), BF)
        for w in range(N_WIN):
            nlo = lo + w * WIN
            nhi = min(lo + (w + 1) * WIN, lo + NODES_PER_CORE)
            a = np.searchsorted(dst_s, nlo); b = np.searchsorted(dst_s, nhi)
            idx = order[a:b]
            n = b - a
            assert n <= ESLOT, f"window overflow {n}"
            s = w * ESLOT
            rinT[:8, s:s + n] = rad[idx].T
            rinT[8:, s:s + n] = attr[idx].T
            nfsT[:, s:s + n] = nf[src[idx]].T
            sh9[s:s + n, :] = sh[idx]
            S[s + np.arange(n), (dst[idx] - nlo)] = BF(1.0)
        nfT = np.zeros((MUL, N_WIN * WIN), BF)
        nfT[:, :NODES_PER_CORE] = nf[lo:lo + NODES_PER_CORE].T.astype(BF)
        rec = np.ones((N_WIN * WIN, 1), np.float32)
        rec[:NODES_PER_CORE, 0] = rec_all[lo:lo + NODES_PER_CORE]
        cores.append(dict(rinT=rinT, nfsT=nfsT, sh9=sh9, S=S, nfT=nfT, rec=rec))
    return cores


def ref_from_kap(x_kap):
    out = np.empty_like(x_kap)
    for l, d in enumerate(DIMS):
        blk = x_kap[:, LOFF[l]:LOFF[l] + 32 * d].reshape(-1, d, 32)
        out[:, LOFF[l]:LOFF[l] + 32 * d] = np.transpose(blk, (0, 2, 1)).reshape(-1, 32 * d)
    return out


# ---------------------------------------------------------------------------
# device kernel
# ---------------------------------------------------------------------------

_NC_CACHE = {}
LAST_RESULT = None

# fraction of product work sent to gpsimd (tuned from profiles)
GP_ELEM_NS = 2.05e-3   # us per elem per partition-row... (us per free-elem)
VE_ELEM_NS = 0.52e-3
GP_OP_OH = 0.25
VE_OP_OH = 0.08


def build_nc(plan):
    import concourse.bass as bass
    import concourse.bacc as bacc
    import concourse.mybir as mybir
    import concourse.tile as tile

    f32 = mybir.dt.float32
    bf16 = mybir.dt.bfloat16
    AL = mybir.AluOpType
    AF = mybir.ActivationFunctionType

    nc = bacc.Bacc(None)
    P = 128

    # ---- dram parameters
    rinT_d = nc.declare_dram_parameter("rinT", [24, E_PAD], f32, isOutput=False)
    nfsT_d = nc.declare_dram_parameter("nfsT", [32, E_PAD], f32, isOutput=False)
    sh9_d = nc.declare_dram_parameter("sh9", [E_PAD, 9], f32, isOutput=False)
    S_d = nc.declare_dram_parameter("S", [E_PAD, 128], bf16, isOutput=False)
    nfT_d = nc.declare_dram_parameter("nfT", [32, N_WIN * WIN], bf16, isOutput=False)
    rec_d = nc.declare_dram_parameter("rec", [N_WIN * WIN, 1], f32, isOutput=False)
    omega_d = nc.declare_dram_parameter("omega", [P, plan.totc], bf16, isOutput=False)
    a2_d = nc.declare_dram_parameter("a2", [P, 16 * 96], bf16, isOutput=False)
    b2_d = nc.declare_dram_parameter("b2", [32, 96], bf16, isOutput=False)
    omc1_d = nc.declare_dram_parameter("omc1", [32, 96], bf16, isOutput=False)
    omself_d = nc.declare_dram_parameter("omself", [32, 32], bf16, isOutput=False)
    rw1_d = nc.declare_dram_parameter("rw1", [24, 64], f32, isOutput=False)
    rw2_d = nc.declare_dram_parameter("rw2", [64, 64], f32, isOutput=False)
    rw3_d = nc.declare_dram_parameter("rw3", [64, 64], f32, isOutput=False)
    rb1_d = nc.declare_dram_parameter("rb1", [64, 1], f32, isOutput=False)
    rb2_d = nc.declare_dram_parameter("rb2", [64, 1], f32, isOutput=False)
    emb_d = nc.declare_dram_parameter("emb", [32, 32], f32, isOutput=False)
    identb_d = nc.declare_dram_parameter("identb", [P, P], bf16, isOutput=False)
    selfull_d = nc.declare_dram_parameter("selfull", [P, 1024], bf16, isOutput=False)
    selr_d = nc.declare_dram_parameter("selr", [64, 2048], bf16, isOutput=False)
    repfull_d = nc.declare_dram_parameter("repfull", [P, P], bf16, isOutput=False)
    zer_d = nc.declare_dram_parameter("zer", [1, P], bf16, isOutput=False)
    zer2_d = nc.declare_dram_parameter("zer2", [1, 2 * HID], bf16, isOutput=False)
    out_d = nc.declare_dram_parameter("out", [N_WIN * WIN, HID], f32, isOutput=True)

    # engine schedule for product ops: greedy balance vector vs gpsimd
    def make_sched():
        ops = []   # (kind, idx, width_elems)
        for gi, (gI, gJ, nJ, b0) in enumerate(plan.aa_groups):
            ops.append(('aa', gi, nJ * 1024))
        for gi, (gI, gJ, nJ, b0) in enumerate(plan.qa_groups):
            ops.append(('qa', gi, nJ * 1024))
        for q in range(4):
            ops.append(('edge', q, 4096))
        v_t, g_t = 1.5, 0.0   # vector starts with msgs/evac budget
        sched = {}
        for (kind, idx, wdt) in ops:
            vc = wdt * VE_ELEM_NS + VE_OP_OH
            gc = wdt * GP_ELEM_NS + GP_OP_OH
            if g_t + gc < v_t + vc:
                sched[(kind, idx)] = 'gpsimd'; g_t += gc
            else:
                sched[(kind, idx)] = 'vector'; v_t += vc
        return sched

    sched = make_sched()

    from contextlib import ExitStack
    with tile.TileContext(nc) as tc, ExitStack() as es:
        cst = es.enter_context(tc.tile_pool(name="cst", bufs=1))
        sb2 = es.enter_context(tc.tile_pool(name="sb2", bufs=2))
        sb3 = es.enter_context(tc.tile_pool(name="sb3", bufs=3))
        uu_pool = es.enter_context(tc.tile_pool(name="uu", bufs=1))
        pt_pool = es.enter_context(tc.tile_pool(name="pt", bufs=2))
        ed_pool = es.enter_context(tc.tile_pool(name="ed", bufs=1))
        sb1 = es.enter_context(tc.tile_pool(name="sb1", bufs=1))
        ps_wps = es.enter_context(tc.tile_pool(name="pswps", bufs=1, space="PSUM"))
        ps_uub = es.enter_context(tc.tile_pool(name="psuub", bufs=2, space="PSUM"))
        ps_qm = es.enter_context(tc.tile_pool(name="psqm", bufs=1, space="PSUM"))
        ps_tp = es.enter_context(tc.tile_pool(name="pstp", bufs=1, space="PSUM"))
        ps_mlp = es.enter_context(tc.tile_pool(name="psmlp", bufs=1, space="PSUM"))
        ps_mx = es.enter_context(tc.tile_pool(name="psmx", bufs=1, space="PSUM"))

        # ---- constants
        omega = cst.tile([P, plan.totc], bf16)
        nc.sync.dma_start(out=omega[:], in_=omega_d[:])
        a2 = cst.tile([P, 16 * 96], bf16)
        nc.sync.dma_start(out=a2[:], in_=a2_d[:])
        b2 = cst.tile([32, 96], bf16); nc.sync.dma_start(out=b2[:], in_=b2_d[:])
        omc1 = cst.tile([32, 96], bf16); nc.sync.dma_start(out=omc1[:], in_=omc1_d[:])
        omself = cst.tile([32, 32], bf16); nc.sync.dma_start(out=omself[:], in_=omself_d[:])
        rw1 = cst.tile([24, 64], f32); nc.sync.dma_start(out=rw1[:], in_=rw1_d[:])
        rw2 = cst.tile([64, 64], f32); nc.sync.dma_start(out=rw2[:], in_=rw2_d[:])
        rw3 = cst.tile([64, 64], f32); nc.sync.dma_start(out=rw3[:], in_=rw3_d[:])
        rb1 = cst.tile([64, 1], f32); nc.sync.dma_start(out=rb1[:], in_=rb1_d[:])
        rb2 = cst.tile([64, 1], f32); nc.sync.dma_start(out=rb2[:], in_=rb2_d[:])
        emb = cst.tile([32, 32], f32); nc.sync.dma_start(out=emb[:], in_=emb_d[:])
        identb = cst.tile([P, P], bf16); nc.sync.dma_start(out=identb[:], in_=identb_d[:])
        selfull = cst.tile([P, 1024], bf16); nc.sync.dma_start(out=selfull[:], in_=selfull_d[:])
        selr = cst.tile([64, 2048], bf16); nc.sync.dma_start(out=selr[:], in_=selr_d[:])
        repfull = cst.tile([P, P], bf16); nc.sync.dma_start(out=repfull[:], in_=repfull_d[:])
        zer = cst.tile([1, P], bf16); nc.sync.dma_start(out=zer[:], in_=zer_d[:])
        zer2 = cst.tile([1, 2 * HID], bf16); nc.sync.dma_start(out=zer2[:], in_=zer2_d[:])
        nfT = cst.tile([32, N_WIN * WIN], bf16)
        nc.sync.dma_start(out=nfT[:], in_=nfT_d[:])

        def transpose3(x_bf, tag):
            """x_bf [128, 288] bf16 -> aT sbuf [128, 384] (chunk-major)."""
            tp = ps_tp.tile([P, 384], bf16, space="PSUM", tag="tp")
            nc.tensor.transpose(out=tp[:, 0:P], in_=x_bf[:, 0:P], identity=identb[:])
            nc.tensor.transpose(out=tp[:, P:2 * P], in_=x_bf[:, P:2 * P], identity=identb[:])
            nc.tensor.transpose(out=tp[0:32, 2 * P:3 * P], in_=x_bf[:, 2 * P:HID], identity=identb[:])
            xt = sb2.tile([P, 384], bf16, tag=tag + "sb")
            nc.scalar.copy(out=xt[:, 0:2 * P], in_=tp[:, 0:2 * P])
            nc.scalar.copy(out=xt[0:32, 2 * P:3 * P], in_=tp[0:32, 2 * P:3 * P])
            return xt

        NCC = [3, 2, 2, 2]   # comps per partition-row-group b: g = 4*cc + b <= 8

        def build_uu(aT, uu_tile, ev):
            """uu[32*u4+v, (g,kc,n)] = aT-val[f=32g+4kc+u4, node n] via SEL matmuls."""
            for b in range(4):
                ncc = NCC[b]
                for kc in range(8):
                    up = ps_uub.tile([P, 512], f32, space="PSUM", tag="uub")
                    nc.tensor.matmul(out=up[:, :ncc * P],
                                     lhsT=selfull[32 * b:32 * (b + 1), kc * P:(kc + 1) * P],
                                     rhs=aT[32 * b:32 * (b + 1), :ncc * P],
                                     start=True, stop=True, tile_position=(32 * b, 0))
                    eng = nc.vector if (ev[0] % 2 == 0) else nc.scalar
                    ev[0] += 1
                    dst = uu_tile[:].rearrange("p (g k n) -> p g k n", k=8, n=P)[:, b::4, kc, :]
                    if eng is nc.vector:
                        eng.tensor_copy(out=dst, in_=up[:, :ncc * P].rearrange("p (c n) -> p c n", n=P))
                    else:
                        eng.copy(out=dst, in_=up[:, :ncc * P].rearrange("p (c n) -> p c n", n=P))

        def build_v8(aT, v8_tile, ev):
            """v8[32*b+v, (g,n)] = aT-val[f=32g+v, node n] (mod-32 replication)."""
            for b in range(4):
                ncc = NCC[b]
                up = ps_uub.tile([P, 512], f32, space="PSUM", tag="uub")
                nc.tensor.matmul(out=up[:, :ncc * P],
                                 lhsT=repfull[32 * b:32 * (b + 1), :],
                                 rhs=aT[32 * b:32 * (b + 1), :ncc * P],
                                 start=True, stop=True, tile_position=(32 * b, 0))
                eng = nc.vector if (ev[0] % 2 == 0) else nc.scalar
                ev[0] += 1
                dst = v8_tile[:].rearrange("p (g n) -> p g n", n=P)[:, b::4, :]
                if eng is nc.vector:
                    eng.tensor_copy(out=dst, in_=up[:, :ncc * P].rearrange("p (c n) -> p c n", n=P))
                else:
                    eng.copy(out=dst, in_=up[:, :ncc * P].rearrange("p (c n) -> p c n", n=P))

        def products_and_emissions(groups, blocks, emi, uu, v8, qm, kind):
            for gi, (gI, gJ, nJ, b0) in enumerate(groups):
                w = nJ * 1024
                PT = pt_pool.tile([P, MAX_JRUN * 1024], bf16, tag="PT")
                eng = nc.gpsimd if sched[(kind, gi)] == 'gpsimd' else nc.vector
                eng.tensor_tensor(
                    out=PT[:, :w].rearrange("p (j k n) -> p j k n", k=8, n=P),
                    in0=uu[:, gI * 1024:(gI + 1) * 1024]
                        .rearrange("p (k n) -> p k n", n=P)[:, None, :, :]
                        .broadcast_to([P, nJ, 8, P]),
                    in1=v8[:, gJ * P:(gJ + nJ) * P]
                        .rearrange("p (j n) -> p j n", n=P)[:, :, None, :]
                        .broadcast_to([P, nJ, 8, P]),
                    op=AL.mult)
                for jl in range(nJ):
                    bi = b0 + jl
                    for kc in range(8):
                        for (c0, c1, off) in emi[bi][kc]:
                            nc.tensor.matmul(out=qm[:, c0:c1],
                                             lhsT=PT[:, jl * 1024 + kc * P: jl * 1024 + (kc + 1) * P],
                                             rhs=omega[:, off:off + (c1 - c0)],
                                             start=False, stop=False,
                                             skip_group_check=True)

        # ---------------- main loop ----------------
        for w in range(N_WIN):
            e0 = w * ESLOT
            # ---- edge phase: radial MLP + embedding, in 2 halves of 512
            rfT = sb1.tile([64, ESLOT], bf16, tag="rfT")
            hT = sb1.tile([32, ESLOT], bf16, tag="hT")
            for h in range(2):
                s = e0 + h * 512
                rin_h = sb2.tile([24, 512], f32, tag="rin")
                nc.sync.dma_start(out=rin_h[:], in_=rinT_d[:, s:s + 512])
                nfs_h = sb2.tile([32, 512], f32, tag="nfs")
                nc.sync.dma_start(out=nfs_h[:], in_=nfsT_d[:, s:s + 512])
                l1p = ps_mlp.tile([64, 512], f32, space="PSUM", tag="mlp")
                nc.tensor.matmul(out=l1p[:], lhsT=rw1[:], rhs=rin_h[:], start=True, stop=True)
                f1 = sb2.tile([64, 512], f32, tag="f")
                nc.scalar.activation(out=f1[:], in_=l1p[:], func=AF.Silu, bias=rb1[:], scale=1.0)
                l2p = ps_mlp.tile([64, 512], f32, space="PSUM", tag="mlp")
                nc.tensor.matmul(out=l2p[:], lhsT=rw2[:], rhs=f1[:], start=True, stop=True)
                f2 = sb2.tile([64, 512], f32, tag="f")
                nc.scalar.activation(out=f2[:], in_=l2p[:], func=AF.Silu, bias=rb2[:], scale=1.0)
                rfp = ps_mlp.tile([64, 512], f32, space="PSUM", tag="mlp")
                nc.tensor.matmul(out=rfp[:], lhsT=rw3[:], rhs=f2[:], start=True, stop=True)
                nc.vector.tensor_copy(out=rfT[:, h * 512:(h + 1) * 512], in_=rfp[:])
                hp = ps_mlp.tile([32, 512], f32, space="PSUM", tag="mlp")
                nc.tensor.matmul(out=hp[:], lhsT=emb[:], rhs=nfs_h[:], start=True, stop=True)
                nc.scalar.copy(out=hT[:, h * 512:(h + 1) * 512], in_=hp[:])
            # V_h: hT replicated mod-32 across partitions (cheap partition-copy DMAs)
            vh = sb1.tile([P, ESLOT], bf16, tag="vh")
            for b in range(4):
                nc.scalar.dma_start(out=vh[32 * b:32 * (b + 1), :], in_=hT[:])

            wps = ps_wps.tile([P, HID], f32, space="PSUM", tag="wps")
            ev = [0]
            for q in range(4):
                eq = e0 + q * 256
                # UU_rf: [128, (c16, e256)] = rfT[4c + p//32, e] via SEL matmuls
                uurf = ed_pool.tile([P, 4096], bf16, tag="uurf")
                for c in range(16):
                    up = ps_uub.tile([P, 512], f32, space="PSUM", tag="uub")
                    nc.tensor.matmul(out=up[:, :256],
                                     lhsT=selr[:, c * P:(c + 1) * P],
                                     rhs=rfT[:, q * 256:(q + 1) * 256],
                                     start=True, stop=True)
                    eng = nc.vector if (ev[0] % 2 == 0) else nc.scalar
                    ev[0] += 1
                    if eng is nc.vector:
                        eng.tensor_copy(out=uurf[:, c * 256:(c + 1) * 256], in_=up[:, :256])
                    else:
                        eng.copy(out=uurf[:, c * 256:(c + 1) * 256], in_=up[:, :256])
                mT = ed_pool.tile([P, 4096], bf16, tag="mT")
                eng = nc.gpsimd if sched[('edge', q)] == 'gpsimd' else nc.vector
                eng.tensor_tensor(
                    out=mT[:].rearrange("p (c e) -> p c e", e=256),
                    in0=uurf[:].rearrange("p (c e) -> p c e", e=256),
                    in1=vh[:, q * 256:(q + 1) * 256][:, None, :].broadcast_to([P, 16, 256]),
                    op=AL.mult)
                for tt in range(2):
                    t = q * 2 + tt
                    et = e0 + t * P
                    # mixed = m @ A2 + h @ B2
                    mxp = ps_mx.tile([P, 96], f32, space="PSUM", tag="mx")
                    for c in range(16):
                        nc.tensor.matmul(out=mxp[:], lhsT=mT[:, c * 256 + tt * P:c * 256 + (tt + 1) * P],
                                         rhs=a2[:, c * 96:(c + 1) * 96],
                                         start=(c == 0), stop=False)
                    nc.tensor.matmul(out=mxp[:], lhsT=hT[:, t * P:(t + 1) * P], rhs=b2[:],
                                     start=False, stop=True)
                    # messages msgs[e, 32 g + u] = sh[e, comp(g)] * mixed[e, l*32+u]
                    sh_t = sb3.tile([P, 9], f32, tag="sht")
                    nc.sync.dma_start(out=sh_t[:], in_=sh9_d[et:et + P, :])
                    msgs = sb3.tile([P, HID], bf16, tag="msgs")
                    for l, d in enumerate(DIMS):
                        nc.vector.tensor_tensor(
                            out=msgs[:, LOFF[l]:LOFF[l] + 32 * d].rearrange("p (i u) -> p i u", u=32),
                            in0=sh_t[:, SOFF[l]:SOFF[l] + d][:, :, None].broadcast_to([P, d, 32]),
                            in1=mxp[:, l * 32:(l + 1) * 32][:, None, :].broadcast_to([P, d, 32]),
                            op=AL.mult)
                    S_t = sb3.tile([P, P], bf16, tag="St")
                    nc.sync.dma_start(out=S_t[:], in_=S_d[et:et + P, :])
                    nc.tensor.matmul(out=wps[:], lhsT=S_t[:], rhs=msgs[:],
                                     start=(t == 0), stop=(t == ET_PER_WIN - 1))

            # ---- node phase
            rec_t = sb2.tile([P, 1], f32, tag="rec")
            nc.sync.dma_start(out=rec_t[:], in_=rec_d[w * P:(w + 1) * P, :])
            a_bf = sb2.tile([P, HID], bf16, tag="abf")
            nc.vector.tensor_scalar_mul(out=a_bf[:], in0=wps[:], scalar1=rec_t[:])
            aT = transpose3(a_bf, "at")
            uu = uu_pool.tile([P, NCOMP * 1024], bf16, tag="uu")
            build_uu(aT, uu, ev)
            v8 = sb2.tile([P, NCOMP * P], bf16, tag="v8")
            build_v8(aT, v8, ev)

            qm = ps_qm.tile([P, 2 * HID], f32, space="PSUM", tag="qm")
            nc.tensor.matmul(out=qm[:, 0:512], lhsT=zer[:], rhs=zer2[:, 0:512], start=True, stop=False, skip_group_check=True)
            nc.tensor.matmul(out=qm[:, 512:576], lhsT=zer[:], rhs=zer2[:, 512:576], start=True, stop=False, skip_group_check=True)

            products_and_emissions(plan.aa_groups, plan.aa, plan.aa_emi, uu, v8, qm, 'aa')

            # q evacuation (strided: even 32-col slots), then q replication
            q_bf = sb2.tile([P, HID], bf16, tag="qbf")
            nc.vector.tensor_copy(
                out=q_bf[:].rearrange("p (g c) -> p g c", c=32),
                in_=qm[:].rearrange("p (g t c) -> p g t c", t=2, c=32)[:, :, 0, :])
            qT = transpose3(q_bf, "qt")
            uuq = uu_pool.tile([P, NCOMP * 1024], bf16, tag="uuq")
            build_uu(qT, uuq, ev)

            products_and_emissions(plan.qa_groups, plan.qa, plan.qa_emi, uuq, v8, qm, 'qa')

            # c1: per-component order-1 linear into msg slots (lhsT = V8 rows 0:32)
            for g in range(NCOMP):
                l = COMP_L[g]
                nc.tensor.matmul(out=qm[:, g * 64 + 32:g * 64 + 64],
                                 lhsT=v8[0:32, g * P:(g + 1) * P],
                                 rhs=omc1[:, l * 32:(l + 1) * 32],
                                 start=False, stop=False, skip_group_check=True)
            # self connection: scalar block = component 0 msg slot
            nc.tensor.matmul(out=qm[:, 32:64], lhsT=nfT[:, w * P:(w + 1) * P],
                             rhs=omself[:], start=False, stop=True,
                             skip_group_check=True)
            # evacuate msg slots (odd 32-col slots)
            out_sb = sb2.tile([P, HID], f32, tag="outsb")
            nc.scalar.copy(
                out=out_sb[:].rearrange("p (g c) -> p g c", c=32),
                in_=qm[:].rearrange("p (g t c) -> p g t c", t=2, c=32)[:, :, 1, :])
            nc.sync.dma_start(out=out_d[w * P:(w + 1) * P, :], in_=out_sb[:])

    nc.finalize()
    return nc


def _get_nc(plan):
    if 'nc' not in _NC_CACHE:
        _NC_CACHE['nc'] = build_nc(plan)
    return _NC_CACHE['nc']


def kernel(**inputs):
    global LAST_RESULT
    from concourse.bass_utils import run_bass_kernel_spmd

    inp = {k: np.asarray(v) for k, v in inputs.items()}
    plan = build_plan()
    W = fold_weights(inp)
    om = pack_omega(plan, W)

    A2 = W['A2'].astype(np.float32)
    a2p = np.zeros((128, 16 * 96), np.float32)
    for c in range(16):
        a2p[:, c * 96:(c + 1) * 96] = A2[c * 128:(c + 1) * 128, :]
    # fold r_b3 into B2 (rf = f2 @ rw3; +b3 contribution is linear in h)
    B2 = W['B2'].astype(np.float64).copy()
    b3 = inp['r_b3'].astype(np.float64)
    for u in range(32):
        B2[u, :] += b3 @ A2[np.arange(RHID) * 32 + u, :].astype(np.float64)

    identb = np.eye(128, dtype=np.float32).astype(BF)
    self = None
    selfull = np.zeros((128, 1024), np.float32)
    for p in range(128):
        for kc in range(8):
            u4 = p % 32 - 4 * kc
            if 0 <= u4 < 4:
                selfull[p, kc * 128 + u4 * 32:kc * 128 + (u4 + 1) * 32] = 1.0
    selr = np.zeros((64, 2048), np.float32)
    for q in range(64):
        c, r4 = divmod(q, 4)
        selr[q, c * 128 + r4 * 32:c * 128 + (r4 + 1) * 32] = 1.0
    repfull = np.zeros((128, 128), np.float32)
    for p in range(128):
        for i in range(128):
            if i % 32 == p % 32:
                repfull[p, i] = 1.0

    shared = dict(
        omega=om,
        a2=a2p.astype(BF), b2=B2.astype(np.float32).astype(BF),
        omc1=W['omc1'].astype(np.float32).astype(BF),
        omself=W['omself'].astype(np.float32).astype(BF),
        rw1=W['r_w1'], rw2=W['r_w2'], rw3=W['r_w3'],
        rb1=W['r_b1'].reshape(64, 1), rb2=W['r_b2'].reshape(64, 1),
        emb=W['emb'].astype(np.float32),
        identb=identb,
        selfull=selfull.astype(BF), selr=selr.astype(BF), repfull=repfull.astype(BF),
        zer=np.zeros((1, 128), BF), zer2=np.zeros((1, 2 * HID), BF),
    )
    cores = pack_edges(inp)
    in_maps = []
    for c in range(N_CORES):
        m = dict(shared)
        m.update(rinT=cores[c]['rinT'], nfsT=cores[c]['nfsT'],
                 sh9=cores[c]['sh9'], S=cores[c]['S'], nfT=cores[c]['nfT'],
                 rec=cores[c]['rec'])
        in_maps.append(m)

    nc = _get_nc(plan)
    res = run_bass_kernel_spmd(nc, in_maps, core_ids=list(range(N_CORES)))
    LAST_RESULT = res
    outs = [res.results[c]['out'][:NODES_PER_CORE] for c in range(N_CORES)]
    out_kap = np.concatenate(outs, axis=0).astype(np.float32)
    return ref_from_kap(out_kap)


if __name__ == "__main__":
    plan = build_plan()
    print(f"aa blocks: {len(plan.aa)}  qa blocks: {len(plan.qa)}")
    print(f"aa groups: {len(plan.aa_groups)}  qa groups: {len(plan.qa_groups)}")
    print(f"omega cols: {plan.totc}  ({plan.totc * 128 * 2 / 1e6:.1f} MB bf16)")
    print(f"emissions per node-tile: {plan.n_emi}")
